# revision 1
# baseline (speedup 1.0000x reference)
"""CrossAttention kernel for 8 Trainium2 NeuronCores.

Reference computation (per batch element b):
    q = ts[b] @ q_w.T + q_b          # [512, 1024]
    k = llm[b] @ k_w.T + k_b         # [2048, 1024]
    v = llm[b] @ v_w.T + v_b         # [2048, 1024]
    per head h (16 heads x 64 dims):
        scores = q_h @ k_h.T / 8     # [512, 2048]
        attn = softmax(scores, -1)
        ctx_h = attn @ v_h           # [512, 64]
    out = ctx @ o_w.T + o_b          # [512, 1024]

Sharding: data-parallel over batch (B=8 -> one element per core), no
collectives.  Per-core kernel keeps everything in "feature-major"
layouts so no on-device transposes are needed:

  QT[j, p]  = q_w @ ts.T  + q_b     (j on partitions, bias per-partition)
  KT[j, s]  = k_w @ llm.T + k_b
  V'[s, j'] = llm @ v_w.T + v_b     (natural layout; bias via broadcast add;
                                     j' = 16 heads x 65 cols, col 64 of each
                                     head block is ones -> softmax denom)
  scoresT_h[s, p] = KT_h.T @ QT_h   (K=64 contraction)
  expT = exp(scoresT / 8)           (no max subtraction: |scores/8| < ~3)
  ctx'_h[0:64, p] = V'_h.T @ expT   (accumulated over s; row 64 = denom)
  ctxT_h = ctx'_h[0:64] * (1/denom) (reciprocal + rank-1 fp32 bcast matmul)
  out[p, j] = ctxT.T @ o_wT + o_b

All matmuls bf16 inputs / fp32 PSUM accumulate.  Host does layout-only
prep (transpose, bf16 cast, bias broadcast).
"""
import numpy as np
import ml_dtypes

D = 1024          # d_model
P = 512           # ts sequence length
S = 2048          # llm sequence length
H = 16            # heads
DH = 64           # head dim
NCORES = 8
NDT = D // 128    # 8 d-tiles
NST = S // 128    # 16 s-tiles
NPT = P // 128    # 4 p-tiles
PS_S, PS_C, PS_P, PIPE_N = 2, 2, 2, 3

_BF16 = ml_dtypes.bfloat16

_cached_nc = None


def _build_nc():
    import concourse.tile as tile
    from concourse import bacc, mybir

    f32 = mybir.dt.float32
    bf16 = mybir.dt.bfloat16
    Exp = mybir.ActivationFunctionType.Exp

    nc = bacc.Bacc("TRN2", target_bir_lowering=False, debug=False,
                   num_devices=NCORES)

    tsT = nc.declare_dram_parameter("tsT", [D, P], bf16, isOutput=False)
    llmT = nc.declare_dram_parameter("llmT", [D, S], bf16, isOutput=False)
    qwT = nc.declare_dram_parameter("qwT", [D, D], bf16, isOutput=False)
    kwT = nc.declare_dram_parameter("kwT", [D, D], bf16, isOutput=False)
    vwT = nc.declare_dram_parameter("vwT", [D, D], bf16, isOutput=False)
    owT = nc.declare_dram_parameter("owT", [D, D], bf16, isOutput=False)
    qkb = nc.declare_dram_parameter("qkb", [128, 2 * NDT], f32, isOutput=False)
    vbb = nc.declare_dram_parameter("vbb", [128, D], f32, isOutput=False)
    obb = nc.declare_dram_parameter("obb", [128, D], f32, isOutput=False)
    out = nc.declare_dram_parameter("out", [P, D], f32, isOutput=True)

    with tile.TileContext(nc) as tc:
        _emit(tc, nc, tile, mybir, f32, bf16, Exp,
              tsT, llmT, qwT, kwT, vwT, owT, qkb, vbb, obb, out)
    nc.compile()
    return nc


def _emit(tc, nc, tile, mybir, f32, bf16, Exp,
          tsT, llmT, qwT, kwT, vwT, owT, qkb, vbb, obb, out):
    from contextlib import ExitStack

    with ExitStack() as ctx:
        persist = ctx.enter_context(tc.tile_pool(name="persist", bufs=1))
        wpool = ctx.enter_context(tc.tile_pool(name="wpool", bufs=24))
        expool = ctx.enter_context(tc.tile_pool(name="expool", bufs=4))
        rpool = ctx.enter_context(tc.tile_pool(name="rpool", bufs=2))
        opool = ctx.enter_context(tc.tile_pool(name="opool", bufs=3))

        # ---- constants / biases ----
        qkb_sb = persist.tile([128, 2 * NDT], f32, name="qkb_sb", tag="qkb_sb")
        nc.sync.dma_start(out=qkb_sb, in_=qkb.ap())
        vbb_sb = persist.tile([128, D], f32, name="vbb_sb", tag="vbb_sb")
        nc.sync.dma_start(out=vbb_sb, in_=vbb.ap())
        obb_sb = persist.tile([128, D], f32, name="obb_sb", tag="obb_sb")
        nc.sync.dma_start(out=obb_sb, in_=obb.ap())
        ones64 = persist.tile([1, DH], bf16, name="ones64", tag="ones64")
        nc.vector.memset(ones64, 1.0)

        # ---- activations + weights, DMA'd in first-use order:
        # QT needs ts+qw only, so those go first; then kw+llm (KT), then
        # vw (V'), then ow (o-proj, slot-waits behind qw/kw anyway).
        def load_w(dram, prefix):
            tiles = []
            for d in range(NDT):
                t = wpool.tile([128, D], bf16, name=f"{prefix}{d}", tag="w")
                nc.sync.dma_start(out=t, in_=dram.ap()[d * 128:(d + 1) * 128, :])
                tiles.append(t)
            return tiles

        ts_sb = []
        for d in range(NDT):
            t = persist.tile([128, P], bf16, name=f"ts_sb{d}", tag=f"ts_sb{d}")
            nc.sync.dma_start(out=t, in_=tsT.ap()[d * 128:(d + 1) * 128, :])
            ts_sb.append(t)
        qw_sb = load_w(qwT, "qw_sb")
        kw_sb = load_w(kwT, "kw_sb")
        llm_sb = []
        for d in range(NDT):
            t = persist.tile([128, S], bf16, name=f"llm_sb{d}", tag=f"llm_sb{d}")
            nc.sync.dma_start(out=t, in_=llmT.ap()[d * 128:(d + 1) * 128, :])
            llm_sb.append(t)
        vw_sb = load_w(vwT, "vw_sb")
        ow_sb = load_w(owT, "ow_sb")

        # ========== wavefront: projections + attention interleaved ==========
        # Emission order drives per-engine priority.  Heads run a
        # scores->exp->ctx pipeline from the very start so ScalarE (exp is
        # the single largest non-PE cost) fills while PE is still producing
        # QT/KT/V'.  QT/KT j-tiles and V' s-tiles are emitted on demand,
        # right before the first head stage that consumes them.
        cx_sb = [None] * NDT
        qt_sb = [None] * NDT
        kt_sb = [None] * NDT
        vp_sb = [None] * NST

        with tc.tile_pool(name="psS", bufs=PS_S, space="PSUM") as psS, \
             tc.tile_pool(name="psC", bufs=PS_C, space="PSUM") as psC, \
             tc.tile_pool(name="psP", bufs=max(PS_P, 1), space="PSUM") as _psP:
            psP = psS if PS_P == 0 else _psP
            ptag = "psS" if PS_P == 0 else "psP"

            def emit_proj(jt):
                # QT[j, p] = q_w @ ts.T + q_b
                ps = psP.tile([128, P], f32, name=f"ps_q{jt}", tag=ptag)
                for d in range(NDT):
                    nc.tensor.matmul(
                        ps, lhsT=qw_sb[d][:, jt * 128:(jt + 1) * 128],
                        rhs=ts_sb[d], start=(d == 0), stop=(d == NDT - 1))
                qt = persist.tile([128, P], bf16, name=f"qt_sb{jt}",
                                  tag=f"qt_sb{jt}")
                nc.vector.tensor_scalar_add(qt, ps, qkb_sb[:, jt:jt + 1])
                qt_sb[jt] = qt
                # KT[j, s] = k_w @ llm.T + k_b
                kt = persist.tile([128, S], bf16, name=f"kt_sb{jt}",
                                  tag=f"kt_sb{jt}")
                for sc in range(S // 512):
                    ps = psP.tile([128, 512], f32, name=f"ps_k{jt}_{sc}",
                                  tag=ptag)
                    for d in range(NDT):
                        nc.tensor.matmul(
                            ps, lhsT=kw_sb[d][:, jt * 128:(jt + 1) * 128],
                            rhs=llm_sb[d][:, sc * 512:(sc + 1) * 512],
                            start=(d == 0), stop=(d == NDT - 1))
                    nc.vector.tensor_scalar_add(
                        kt[:, sc * 512:(sc + 1) * 512], ps,
                        qkb_sb[:, NDT + jt:NDT + jt + 1])
                kt_sb[jt] = kt

            def emit_v(st):
                # V'[s, h*65 + x]: x<64 -> v_h columns, x=64 -> ones
                vp = persist.tile([128, H * (DH + 1)], bf16,
                                  name=f"vp_sb{st}", tag=f"vp_sb{st}")
                vp3 = vp.rearrange("p (h x) -> p h x", x=DH + 1)
                nc.vector.memset(vp3[:, :, DH:DH + 1], 1.0)
                for jc in range(2):
                    ps = psP.tile([128, 512], f32, name=f"ps_v{st}_{jc}",
                                  tag=ptag)
                    for d in range(NDT):
                        nc.tensor.matmul(
                            ps, lhsT=llm_sb[d][:, st * 128:(st + 1) * 128],
                            rhs=vw_sb[d][:, jc * 512:(jc + 1) * 512],
                            start=(d == 0), stop=(d == NDT - 1))
                    nc.vector.tensor_add(
                        vp3[:, jc * 8:(jc + 1) * 8, 0:DH],
                        ps.rearrange("p (h x) -> p h x", x=DH),
                        vbb_sb[:, jc * 512:(jc + 1) * 512]
                        .rearrange("p (h x) -> p h x", x=DH))
                vp_sb[st] = vp

            # Head PAIRS (2p, 2p+1) advance together through 8 stages of
            # 2 s-tiles each.  The two heads' K=64 score matmuls use
            # disjoint PE row groups (partitions 0:64 vs 64:128); issuing
            # them back-to-back lets the PE array run both concurrently.
            # psS 2x2 + psC 2 + psP 2 = 8 PSUM banks.
            emitted_v = [0]

            def ensure_v(upto):
                while emitted_v[0] <= upto:
                    emit_v(emitted_v[0])
                    emitted_v[0] += 1

            for p in range(H // 2):
                jt = p
                if qt_sb[jt] is None:
                    emit_proj(jt)
                psc = [psC.tile([128, 512], f32, name=f"ps_c{2*p+u}",
                                tag="psC") for u in range(2)]
                for k in range(8):
                    pss = [psS.tile([128, 1024], f32,
                                    name=f"ps_s{2*p+u}_{k}", tag="psS")
                           for u in range(2)]
                    for i in range(2):
                        st = 2 * k + i
                        for u in range(2):
                            rs = u * DH
                            nc.tensor.matmul(
                                pss[u][:, i * 512:(i + 1) * 512],
                                lhsT=kt_sb[jt][rs:rs + DH,
                                               st * 128:(st + 1) * 128],
                                rhs=qt_sb[jt][rs:rs + DH, :],
                                start=True, stop=True)
                    ets = []
                    for u in range(2):
                        et = expool.tile([128, 1024], bf16,
                                         name=f"et{2*p+u}_{k}", tag="et")
                        nc.scalar.activation(et, pss[u], Exp,
                                             bias=0.0, scale=0.125)
                        ets.append(et)
                    ensure_v(2 * k + 1)
                    for i in range(2):
                        st = 2 * k + i
                        for u in range(2):
                            h = 2 * p + u
                            nc.tensor.matmul(
                                psc[u][0:DH + 1, :],
                                lhsT=vp_sb[st][:, h * (DH + 1):
                                               (h + 1) * (DH + 1)],
                                rhs=ets[u][:, i * 512:(i + 1) * 512],
                                start=(st == 0), stop=(st == NST - 1))
                for u in range(2):
                    h = 2 * p + u
                    rs = u * DH
                    # normalize: recip of denom row, rank-1 bf16 broadcast
                    # into the unused upper half of the ctx bank, SBUF
                    # staging copy (DVE reads max one PSUM operand), mul.
                    rc = rpool.tile([1, 512], bf16, name=f"rc{h}", tag="rc")
                    with nc.allow_low_precision(
                            reason="bf16 softmax-denominator reciprocal; "
                                   "feeds a bf16 rank-1 broadcast matmul"):
                        nc.vector.reciprocal(rc, psc[u][DH:DH + 1, :])
                    nc.tensor.matmul(psc[u][DH:DH + DH, :], lhsT=ones64,
                                     rhs=rc, start=True, stop=True)
                    rb = rpool.tile([DH, 512], f32, name=f"rb{h}", tag="rb")
                    nc.vector.tensor_copy(rb, psc[u][DH:DH + DH, :])
                    if rs == 0:
                        cx_sb[jt] = persist.tile([128, P], bf16,
                                                 name=f"cx_sb{jt}",
                                                 tag=f"cx_sb{jt}")
                    nc.vector.tensor_mul(cx_sb[jt][rs:rs + DH, :],
                                         psc[u][0:DH, :], rb)

        # ================= phase C: output projection =================
        with tc.tile_pool(name="psO", bufs=4, space="PSUM") as psO:
            for pt in range(NPT):
                for jc in range(2):
                    ps = psO.tile([128, 512], f32, name=f"ps_o{pt}_{jc}",
                                  tag="psO")
                    for d in range(NDT):
                        nc.tensor.matmul(
                            ps, lhsT=cx_sb[d][:, pt * 128:(pt + 1) * 128],
                            rhs=ow_sb[d][:, jc * 512:(jc + 1) * 512],
                            start=(d == 0), stop=(d == NDT - 1))
                    ot = opool.tile([128, 512], f32, name=f"ot{pt}_{jc}",
                                    tag="ot")
                    nc.vector.tensor_add(ot, ps, obb_sb[:, jc * 512:(jc + 1) * 512])
                    nc.sync.dma_start(
                        out=out.ap()[pt * 128:(pt + 1) * 128,
                                     jc * 512:(jc + 1) * 512],
                        in_=ot)


def get_nc():
    global _cached_nc
    if _cached_nc is None:
        _cached_nc = _build_nc()
    return _cached_nc


def make_in_maps(ts_features, llm_features, q_w, q_b, k_w, k_b, v_w, v_b,
                 o_w, o_b):
    ts = np.asarray(ts_features, np.float32)
    llm = np.asarray(llm_features, np.float32)
    shared = {
        "qwT": np.ascontiguousarray(np.asarray(q_w, np.float32).T).astype(_BF16),
        "kwT": np.ascontiguousarray(np.asarray(k_w, np.float32).T).astype(_BF16),
        "vwT": np.ascontiguousarray(np.asarray(v_w, np.float32).T).astype(_BF16),
        "owT": np.ascontiguousarray(np.asarray(o_w, np.float32).T).astype(_BF16),
        "qkb": np.ascontiguousarray(np.concatenate(
            [np.asarray(q_b, np.float32).reshape(NDT, 128).T,
             np.asarray(k_b, np.float32).reshape(NDT, 128).T], axis=1)),
        "vbb": np.ascontiguousarray(
            np.broadcast_to(np.asarray(v_b, np.float32), (128, D))),
        "obb": np.ascontiguousarray(
            np.broadcast_to(np.asarray(o_b, np.float32), (128, D))),
    }
    in_maps = []
    for b in range(NCORES):
        m = dict(shared)
        m["tsT"] = np.ascontiguousarray(ts[b].T).astype(_BF16)
        m["llmT"] = np.ascontiguousarray(llm[b].T).astype(_BF16)
        in_maps.append(m)
    return in_maps


def kernel(**inputs):
    from concourse.bass_utils import run_bass_kernel_spmd

    nc = get_nc()
    in_maps = make_in_maps(**inputs)
    res = run_bass_kernel_spmd(nc, in_maps, list(range(NCORES)))
    return np.stack([res.results[i]["out"] for i in range(NCORES)], axis=0)



# revision 4
# speedup vs baseline: 1.1866x; 1.1866x over previous
"""CrossAttention kernel for 8 Trainium2 NeuronCores.

Reference computation (per batch element b):
    q = ts[b] @ q_w.T + q_b          # [512, 1024]
    k = llm[b] @ k_w.T + k_b         # [2048, 1024]
    v = llm[b] @ v_w.T + v_b         # [2048, 1024]
    per head h (16 heads x 64 dims):
        scores = q_h @ k_h.T / 8     # [512, 2048]
        attn = softmax(scores, -1)
        ctx_h = attn @ v_h           # [512, 64]
    out = ctx @ o_w.T + o_b          # [512, 1024]

Sharding: data-parallel over batch (B=8 -> one element per core), no
collectives.

Per-core layout strategy (cost model: matmul time = out-free-size only,
so maximize output partitions and minimize re-streamed rows):

  QT[j, p]  = q_w @ ts.T  + q_b       (feature-major, bias per-partition)
  KT[j, s]  = k_w @ llm.T + k_b
  V'[s, j'] = llm @ v_w.T + v_b       (natural layout; j' = 16 heads x 65
                                       cols, col 64 of each head block is
                                       ones -> softmax denominator)
  scoresT_h[s, p] = KT_h.T @ QT_h     (K=64 contraction, N=512)
  expT = exp(scoresT / 8)             (no max subtraction: |scores/8| < ~3)
  ctx_h[p, 0:65] = expT_pt.T @ V'_h   (N=65 matmuls accumulated over s;
                                       col 64 = softmax denominator)
  normalize: rc = 1/ctx[:, 64];  ctx_pd[pt][:, h*64:+64] = ctx[:,0:64]*rc
                                      (per-partition scalar -> cheap DVE)
  cxT[jt] = dma-xbar-transpose(ctx_pd)  (SBUF->SBUF on idle DMA engines)
  out[p, j] = cxT.T @ o_wT + o_b

All matmuls bf16 inputs / fp32 PSUM accumulate.  Host does layout-only
prep (transpose, bf16 cast, bias broadcast).
"""
import numpy as np
import ml_dtypes

D = 1024          # d_model
P = 512           # ts sequence length
S = 2048          # llm sequence length
H = 16            # heads
DH = 64           # head dim
NCORES = 8
NDT = D // 128    # 8 d-tiles
NST = S // 128    # 16 s-tiles
NPT = P // 128    # 4 p-tiles
NG = NST // 2     # 8 score groups per head (2 s-tiles each)

_BF16 = ml_dtypes.bfloat16

_cached_nc = None


def _build_nc():
    import concourse.tile as tile
    from concourse import bacc, mybir

    f32 = mybir.dt.float32
    bf16 = mybir.dt.bfloat16
    Exp = mybir.ActivationFunctionType.Exp

    nc = bacc.Bacc("TRN2", target_bir_lowering=False, debug=False,
                   num_devices=NCORES)

    tsT = nc.declare_dram_parameter("tsT", [D, P], bf16, isOutput=False)
    llmT = nc.declare_dram_parameter("llmT", [D, S], bf16, isOutput=False)
    qwT = nc.declare_dram_parameter("qwT", [D, D], bf16, isOutput=False)
    kwT = nc.declare_dram_parameter("kwT", [D, D], bf16, isOutput=False)
    vwT = nc.declare_dram_parameter("vwT", [D, D], bf16, isOutput=False)
    owT = nc.declare_dram_parameter("owT", [D, D], bf16, isOutput=False)
    qkb = nc.declare_dram_parameter("qkb", [128, 2 * NDT], f32, isOutput=False)
    vbb = nc.declare_dram_parameter("vbb", [128, D], bf16, isOutput=False)
    obb = nc.declare_dram_parameter("obb", [128, D], f32, isOutput=False)
    out = nc.declare_dram_parameter("out", [P, D], f32, isOutput=True)

    with tile.TileContext(nc) as tc:
        _emit(tc, nc, tile, mybir, f32, bf16, Exp,
              tsT, llmT, qwT, kwT, vwT, owT, qkb, vbb, obb, out)
    nc.compile()
    return nc


def _emit(tc, nc, tile, mybir, f32, bf16, Exp,
          tsT, llmT, qwT, kwT, vwT, owT, qkb, vbb, obb, out):
    from contextlib import ExitStack

    with ExitStack() as ctx:
        persist = ctx.enter_context(tc.tile_pool(name="persist", bufs=1))
        wpool = ctx.enter_context(tc.tile_pool(name="wpool", bufs=20))
        expool = ctx.enter_context(tc.tile_pool(name="expool", bufs=6))
        rpool = ctx.enter_context(tc.tile_pool(name="rpool", bufs=4))
        opool = ctx.enter_context(tc.tile_pool(name="opool", bufs=3))

        # ---- DMAs in first-use order.  qkb tiny; ts+qw feed QT (first PE
        # work); kw+llm feed KT; vw+vbb feed V'; ow+obb feed o-proj (last).
        qkb_sb = persist.tile([128, 2 * NDT], f32, name="qkb_sb", tag="qkb_sb")
        nc.sync.dma_start(out=qkb_sb, in_=qkb.ap())

        ts_sb = []
        for d in range(NDT):
            t = persist.tile([128, P], bf16, name=f"ts_sb{d}", tag=f"ts_sb{d}")
            nc.sync.dma_start(out=t, in_=tsT.ap()[d * 128:(d + 1) * 128, :])
            ts_sb.append(t)

        def load_w(dram, prefix):
            tiles = []
            for d in range(NDT):
                t = wpool.tile([128, D], bf16, name=f"{prefix}{d}", tag="w")
                nc.sync.dma_start(out=t, in_=dram.ap()[d * 128:(d + 1) * 128, :])
                tiles.append(t)
            return tiles

        qw_sb = load_w(qwT, "qw_sb")
        kw_sb = load_w(kwT, "kw_sb")
        llm_sb = []
        for d in range(NDT):
            t = persist.tile([128, S], bf16, name=f"llm_sb{d}", tag=f"llm_sb{d}")
            nc.sync.dma_start(out=t, in_=llmT.ap()[d * 128:(d + 1) * 128, :])
            llm_sb.append(t)
        vw_sb = load_w(vwT, "vw_sb")
        vbb_sb = persist.tile([128, D], bf16, name="vbb_sb", tag="vbb_sb")
        nc.sync.dma_start(out=vbb_sb, in_=vbb.ap())
        ow_sb = load_w(owT, "ow_sb")
        obb_sb = persist.tile([128, D], f32, name="obb_sb", tag="obb_sb")
        nc.sync.dma_start(out=obb_sb, in_=obb.ap())

        # ---- persistent activation/result tiles ----
        qt_sb = [None] * NDT
        kt_sb = [None] * NDT
        vp_sb = [None] * NST
        cx_sb = [persist.tile([128, P], bf16, name=f"cx_sb{jt}",
                              tag=f"cx_sb{jt}") for jt in range(NDT)]
        ctx_pd = [persist.tile([128, D], bf16, name=f"ctx_pd{pt}",
                               tag=f"ctx_pd{pt}") for pt in range(NPT)]

        with tc.tile_pool(name="psS", bufs=2, space="PSUM") as psS, \
             tc.tile_pool(name="psC", bufs=2, space="PSUM") as psC, \
             tc.tile_pool(name="psV", bufs=2, space="PSUM") as psV:

            def emit_qt(jt):
                # QT[j, p] = q_w @ ts.T + q_b
                ps = psV.tile([128, P], f32, name=f"ps_q{jt}", tag="psV")
                for d in range(NDT):
                    nc.tensor.matmul(
                        ps, lhsT=qw_sb[d][:, jt * 128:(jt + 1) * 128],
                        rhs=ts_sb[d], start=(d == 0), stop=(d == NDT - 1))
                qt = persist.tile([128, P], bf16, name=f"qt_sb{jt}",
                                  tag=f"qt_sb{jt}")
                nc.vector.tensor_scalar_add(qt, ps, qkb_sb[:, jt:jt + 1])
                qt_sb[jt] = qt

            def emit_kt(jt):
                # KT[j, s] = k_w @ llm.T + k_b
                kt = persist.tile([128, S], bf16, name=f"kt_sb{jt}",
                                  tag=f"kt_sb{jt}")
                for sc in range(S // 512):
                    ps = psV.tile([128, 512], f32, name=f"ps_k{jt}_{sc}",
                                  tag="psV")
                    for d in range(NDT):
                        nc.tensor.matmul(
                            ps, lhsT=kw_sb[d][:, jt * 128:(jt + 1) * 128],
                            rhs=llm_sb[d][:, sc * 512:(sc + 1) * 512],
                            start=(d == 0), stop=(d == NDT - 1))
                    nc.vector.tensor_scalar_add(
                        kt[:, sc * 512:(sc + 1) * 512], ps,
                        qkb_sb[:, NDT + jt:NDT + jt + 1])
                kt_sb[jt] = kt

            def emit_v(st):
                # V'[s, h*65 + x]: x<64 -> v_h columns, x=64 -> ones
                vp = persist.tile([128, H * (DH + 1)], bf16,
                                  name=f"vp_sb{st}", tag=f"vp_sb{st}")
                vp3 = vp.rearrange("p (h x) -> p h x", x=DH + 1)
                nc.vector.memset(vp3[:, :, DH:DH + 1], 1.0)
                for jc in range(2):
                    ps = psV.tile([128, 512], f32, name=f"ps_v{st}_{jc}",
                                  tag="psV")
                    for d in range(NDT):
                        nc.tensor.matmul(
                            ps, lhsT=llm_sb[d][:, st * 128:(st + 1) * 128],
                            rhs=vw_sb[d][:, jc * 512:(jc + 1) * 512],
                            start=(d == 0), stop=(d == NDT - 1))
                    nc.vector.tensor_add(
                        vp3[:, jc * 8:(jc + 1) * 8, 0:DH],
                        ps.rearrange("p (h x) -> p h x", x=DH),
                        vbb_sb[:, jc * 512:(jc + 1) * 512]
                        .rearrange("p (h x) -> p h x", x=DH))
                vp_sb[st] = vp

            emitted_v = [0]

            def ensure_v(upto):
                while emitted_v[0] <= upto:
                    emit_v(emitted_v[0])
                    emitted_v[0] += 1

            # QT is the only work available while kw/llm still stream in.
            for jt in range(NDT):
                emit_qt(jt)

            # ---- heads, one at a time ----
            for h in range(H):
                jt = h // 2
                rs = (h % 2) * DH
                if kt_sb[jt] is None:
                    emit_kt(jt)
                # ctx accumulator: 4 p-tiles x (64 ctx cols + denom col).
                # All 4 groups share one PSUM bank, and a matmul with
                # start=True zeroes the whole bank -- so zero the tile once
                # and accumulate with start=False in every matmul.
                psc = psC.tile([128, NPT * (DH + 1)], f32, name=f"ps_c{h}",
                               tag="psC")
                nc.vector.memset(psc, 0.0)
                for g in range(NG):
                    pss = psS.tile([128, 1024], f32, name=f"ps_s{h}_{g}",
                                   tag="psS")
                    for i in range(2):
                        st = 2 * g + i
                        nc.tensor.matmul(
                            pss[:, i * 512:(i + 1) * 512],
                            lhsT=kt_sb[jt][rs:rs + DH,
                                           st * 128:(st + 1) * 128],
                            rhs=qt_sb[jt][rs:rs + DH, :],
                            start=True, stop=True)
                    et = expool.tile([128, 1024], bf16, name=f"et{h}_{g}",
                                     tag="et")
                    nc.scalar.activation(et, pss, Exp, bias=0.0, scale=0.125)
                    ensure_v(2 * g + 1)
                    for i in range(2):
                        st = 2 * g + i
                        for pt in range(NPT):
                            nc.tensor.matmul(
                                psc[:, pt * (DH + 1):(pt + 1) * (DH + 1)],
                                lhsT=et[:, i * 512 + pt * 128:
                                        i * 512 + (pt + 1) * 128],
                                rhs=vp_sb[st][:, h * (DH + 1):
                                              (h + 1) * (DH + 1)],
                                start=False, stop=(st == NST - 1))
                # normalize: per-partition reciprocal of the denominator
                # column, then scale the 64 ctx columns into ctx_pd.
                for pt in range(NPT):
                    c0 = pt * (DH + 1)
                    rc = rpool.tile([128, 1], f32, name=f"rc{h}_{pt}",
                                    tag="rc")
                    nc.vector.reciprocal(rc, psc[:, c0 + DH:c0 + DH + 1])
                    nc.vector.tensor_scalar_mul(
                        ctx_pd[pt][:, h * DH:(h + 1) * DH],
                        psc[:, c0:c0 + DH], rc)
                if h % 2 == 1:
                    # both heads of d-tile jt done: transpose [p,d]->[d,p]
                    # on the DMA xbar (SBUF->SBUF, off the PE critical path)
                    for pt in range(NPT):
                        nc.sync.dma_start_transpose(
                            out=cx_sb[jt][:, pt * 128:(pt + 1) * 128],
                            in_=ctx_pd[pt][:, jt * 128:(jt + 1) * 128])

            # ================= output projection =================
            for pt in range(NPT):
                for jc in range(2):
                    ps = psV.tile([128, 512], f32, name=f"ps_o{pt}_{jc}",
                                  tag="psV")
                    for d in range(NDT):
                        nc.tensor.matmul(
                            ps, lhsT=cx_sb[d][:, pt * 128:(pt + 1) * 128],
                            rhs=ow_sb[d][:, jc * 512:(jc + 1) * 512],
                            start=(d == 0), stop=(d == NDT - 1))
                    ot = opool.tile([128, 512], f32, name=f"ot{pt}_{jc}",
                                    tag="ot")
                    nc.vector.tensor_add(ot, ps,
                                         obb_sb[:, jc * 512:(jc + 1) * 512])
                    nc.sync.dma_start(
                        out=out.ap()[pt * 128:(pt + 1) * 128,
                                     jc * 512:(jc + 1) * 512],
                        in_=ot)


def get_nc():
    global _cached_nc
    if _cached_nc is None:
        _cached_nc = _build_nc()
    return _cached_nc


def make_in_maps(ts_features, llm_features, q_w, q_b, k_w, k_b, v_w, v_b,
                 o_w, o_b):
    ts = np.asarray(ts_features, np.float32)
    llm = np.asarray(llm_features, np.float32)
    shared = {
        "qwT": np.ascontiguousarray(np.asarray(q_w, np.float32).T).astype(_BF16),
        "kwT": np.ascontiguousarray(np.asarray(k_w, np.float32).T).astype(_BF16),
        "vwT": np.ascontiguousarray(np.asarray(v_w, np.float32).T).astype(_BF16),
        "owT": np.ascontiguousarray(np.asarray(o_w, np.float32).T).astype(_BF16),
        "qkb": np.ascontiguousarray(np.concatenate(
            [np.asarray(q_b, np.float32).reshape(NDT, 128).T,
             np.asarray(k_b, np.float32).reshape(NDT, 128).T], axis=1)),
        "vbb": np.ascontiguousarray(
            np.broadcast_to(np.asarray(v_b, np.float32), (128, D))).astype(_BF16),
        "obb": np.ascontiguousarray(
            np.broadcast_to(np.asarray(o_b, np.float32), (128, D))),
    }
    in_maps = []
    for b in range(NCORES):
        m = dict(shared)
        m["tsT"] = np.ascontiguousarray(ts[b].T).astype(_BF16)
        m["llmT"] = np.ascontiguousarray(llm[b].T).astype(_BF16)
        in_maps.append(m)
    return in_maps


def kernel(**inputs):
    from concourse.bass_utils import run_bass_kernel_spmd

    nc = get_nc()
    in_maps = make_in_maps(**inputs)
    res = run_bass_kernel_spmd(nc, in_maps, list(range(NCORES)))
    return np.stack([res.results[i]["out"] for i in range(NCORES)], axis=0)


# revision 8
# speedup vs baseline: 1.2103x; 1.0200x over previous
"""CrossAttention kernel for 8 Trainium2 NeuronCores.

Reference computation (per batch element b):
    q = ts[b] @ q_w.T + q_b          # [512, 1024]
    k = llm[b] @ k_w.T + k_b         # [2048, 1024]
    v = llm[b] @ v_w.T + v_b         # [2048, 1024]
    per head h (16 heads x 64 dims):
        scores = q_h @ k_h.T / 8     # [512, 2048]
        attn = softmax(scores, -1)
        ctx_h = attn @ v_h           # [512, 64]
    out = ctx @ o_w.T + o_b          # [512, 1024]

Sharding: data-parallel over batch (B=8 -> one element per core), no
collectives.

Per-core layout strategy (cost model: matmul time ~ out-free-size only,
so maximize output partitions and minimize re-streamed rows):

  QT[j, p]  = q_w @ ts.T  + q_b       (feature-major, bias per-partition)
  KT[j, s]  = k_w @ llm.T + k_b
  V'[s, j'] = llm @ v_w.T + v_b       (natural layout; j' = 16 heads x 65
                                       cols, col 64 of each head block is
                                       ones -> softmax denominator)
  scoresT_h[s, p] = KT_h.T @ QT_h     (K=64 contraction, N=512)
  expT = exp(scoresT / 8)             (no max subtraction: |scores/8| < ~3)
  ctx_h[p, 0:65] = expT_pt.T @ V'_h   (N=65 matmuls accumulated over s;
                                       col 64 = softmax denominator)
  normalize: rc = 1/ctx[:, 64];  ctx_pd[pt][:, h*64:+64] = ctx[:,0:64]*rc
                                      (per-partition scalar -> cheap DVE)
  cxT[jt] = transpose(ctx_pd)         (DMA xbar for jt<7, PE transpose +
                                       DVE copy for jt7 to cut latency)
  out[p, j] = cxT.T @ o_wT + o_b      (split: d-tiles 0..5 pre-accumulated
                                       into bf16 partials during the
                                       ACT-bound last heads; d 6..7 + add
                                       in a short tail)

Schedule: PE warms its p-state on dummy matmuls during the initial DMA
wait; QT runs d-major in two 4-group waves (one-tile arrival buffer) so
the q_w stream feeds it continuously; KT starts as a 6-group d-major
wave gated on the llm stream (bias adds on the idle ACT engine); the
remaining KT column chunks are spread one per head as mid-head PE
filler under the ACT-bound exp stream.

All matmuls bf16 inputs / fp32 PSUM accumulate.  Host does layout-only
prep (transpose, bf16 cast, bias broadcast, identity matrix).
"""
import numpy as np
import ml_dtypes

D = 1024          # d_model
P = 512           # ts sequence length
S = 2048          # llm sequence length
H = 16            # heads
DH = 64           # head dim
NCORES = 8
NDT = D // 128    # 8 d-tiles
NST = S // 128    # 16 s-tiles
NPT = P // 128    # 4 p-tiles
NG = NST // 2     # 8 score groups per head (2 s-tiles each)
NWARM = 64        # PE p-state warmup matmuls during initial DMA wait

# head -> (jt, [sc...]) remaining KT chunk emitted inside that head
KT_CHUNKS = {
    0: (1, (2, 3)), 1: (2, (0, 1)), 2: (2, (2, 3)), 3: (3, (0, 1)),
    4: (3, (2, 3)), 5: (4, (0, 1)), 6: (4, (2, 3)), 7: (5, (0, 1)),
    8: (5, (2, 3)), 9: (6, (0, 1)), 10: (6, (2, 3)), 11: (7, (0, 1)),
    12: (7, (2, 3)),
}
# head -> o-proj (pt, jc) groups whose d-tiles 0..5 are pre-accumulated
# as mid-head filler (the last heads have no KT chunks left)
OP1 = {12: ((0, 0),), 13: ((0, 1), (1, 0), (1, 1)),
       14: ((2, 0), (2, 1)), 15: ((3, 0), (3, 1))}

_BF16 = ml_dtypes.bfloat16

_cached_nc = None


def _build_nc():
    import concourse.tile as tile
    from concourse import bacc, mybir

    f32 = mybir.dt.float32
    bf16 = mybir.dt.bfloat16
    Exp = mybir.ActivationFunctionType.Exp
    Ident = mybir.ActivationFunctionType.Identity

    nc = bacc.Bacc("TRN2", target_bir_lowering=False, debug=False,
                   num_devices=NCORES)

    ts2 = nc.declare_dram_parameter("ts2", [128, NDT * P], bf16,
                                    isOutput=False)
    llmT = nc.declare_dram_parameter("llmT", [D, S], bf16, isOutput=False)
    qwT = nc.declare_dram_parameter("qwT", [D, D], bf16, isOutput=False)
    kwT = nc.declare_dram_parameter("kwT", [D, D], bf16, isOutput=False)
    vwT = nc.declare_dram_parameter("vwT", [D, D], bf16, isOutput=False)
    owT = nc.declare_dram_parameter("owT", [D, D], bf16, isOutput=False)
    qkb = nc.declare_dram_parameter("qkb", [128, 2 * NDT], f32, isOutput=False)
    idm = nc.declare_dram_parameter("idm", [128, 128], bf16, isOutput=False)
    vbb = nc.declare_dram_parameter("vbb", [128, D], bf16, isOutput=False)
    obb = nc.declare_dram_parameter("obb", [128, D], f32, isOutput=False)
    out = nc.declare_dram_parameter("out", [P, D], f32, isOutput=True)

    with tile.TileContext(nc) as tc:
        _emit(tc, nc, tile, mybir, f32, bf16, Exp, Ident,
              ts2, llmT, qwT, kwT, vwT, owT, qkb, idm, vbb, obb, out)
    nc.compile()
    return nc


def _emit(tc, nc, tile, mybir, f32, bf16, Exp, Ident,
          ts2, llmT, qwT, kwT, vwT, owT, qkb, idm, vbb, obb, out):
    from contextlib import ExitStack

    with ExitStack() as ctx:
        persist = ctx.enter_context(tc.tile_pool(name="persist", bufs=1))
        wpool = ctx.enter_context(tc.tile_pool(name="wpool", bufs=20))
        expool = ctx.enter_context(tc.tile_pool(name="expool", bufs=6))
        rpool = ctx.enter_context(tc.tile_pool(name="rpool", bufs=4))
        opool = ctx.enter_context(tc.tile_pool(name="opool", bufs=3))

        # ---- DMAs in first-use order ----
        qkb_sb = persist.tile([128, 2 * NDT], f32, name="qkb_sb", tag="qkb_sb")
        nc.sync.dma_start(out=qkb_sb, in_=qkb.ap())
        idm_sb = persist.tile([128, 128], bf16, name="idm_sb", tag="idm_sb")
        nc.sync.dma_start(out=idm_sb, in_=idm.ap())

        def load_w(dram, prefix):
            tiles = []
            for d in range(NDT):
                t = wpool.tile([128, D], bf16, name=f"{prefix}{d}", tag="w")
                nc.sync.dma_start(out=t, in_=dram.ap()[d * 128:(d + 1) * 128, :])
                tiles.append(t)
            return tiles

        # ts in two halves (batched: 8 small DMAs -> 2), interleaved with
        # the q_w tiles that the d-major QT waves consume in order.
        ts_sb = []
        for half in range(2):
            t = persist.tile([128, 4 * P], bf16, name=f"ts_sb{half}",
                             tag=f"ts_sb{half}")
            ts_sb.append(t)
        qw_sb = []
        nc.sync.dma_start(out=ts_sb[0], in_=ts2.ap()[:, 0:4 * P])
        for d in range(4):
            t = wpool.tile([128, D], bf16, name=f"qw_sb{d}", tag="w")
            nc.sync.dma_start(out=t, in_=qwT.ap()[d * 128:(d + 1) * 128, :])
            qw_sb.append(t)
        nc.sync.dma_start(out=ts_sb[1], in_=ts2.ap()[:, 4 * P:8 * P])
        for d in range(4, NDT):
            t = wpool.tile([128, D], bf16, name=f"qw_sb{d}", tag="w")
            nc.sync.dma_start(out=t, in_=qwT.ap()[d * 128:(d + 1) * 128, :])
            qw_sb.append(t)

        kw_sb = load_w(kwT, "kw_sb")
        llm_sb = []
        for d in range(NDT):
            t = persist.tile([128, S], bf16, name=f"llm_sb{d}", tag=f"llm_sb{d}")
            nc.sync.dma_start(out=t, in_=llmT.ap()[d * 128:(d + 1) * 128, :])
            llm_sb.append(t)
        vw_sb = load_w(vwT, "vw_sb")
        vbb_sb = persist.tile([128, D], bf16, name="vbb_sb", tag="vbb_sb")
        nc.sync.dma_start(out=vbb_sb, in_=vbb.ap())
        ow_sb = load_w(owT, "ow_sb")
        obb_sb = persist.tile([128, D], f32, name="obb_sb", tag="obb_sb")
        nc.sync.dma_start(out=obb_sb, in_=obb.ap())

        def ts_d(d):
            return ts_sb[d // 4][:, (d % 4) * P:(d % 4 + 1) * P]

        # ---- persistent activation/result tiles ----
        qt_sb = [None] * NDT
        kt_sb = [None] * NDT
        vp_sb = [None] * NST
        cx_sb = [persist.tile([128, P], bf16, name=f"cx_sb{jt}",
                              tag=f"cx_sb{jt}") for jt in range(NDT)]
        ctx_pd = [persist.tile([128, D], bf16, name=f"ctx_pd{pt}",
                               tag=f"ctx_pd{pt}") for pt in range(NPT)]
        # bf16 o-proj partials (d-tiles 0..5 + o_b)
        otp = {(pt, jc): persist.tile([128, 512], bf16,
                                      name=f"otp{pt}_{jc}",
                                      tag=f"otp{pt}_{jc}")
               for pt in range(NPT) for jc in range(2)}

        with tc.tile_pool(name="psS", bufs=2, space="PSUM") as psS, \
             tc.tile_pool(name="psC", bufs=2, space="PSUM") as psC, \
             tc.tile_pool(name="psV", bufs=2, space="PSUM") as psV:

            # ---- PE p-state warmup on qkb during the initial DMA wait ----
            wps = psV.tile([128, 16], f32, name="ps_warm", tag="psV")
            for i in range(NWARM):
                nc.tensor.matmul(wps[0:16, :], lhsT=qkb_sb[:, 0:16],
                                 rhs=qkb_sb, start=True, stop=True)

            # ======== QT: d-major, two waves of 4 concurrent j-groups ====
            # (each arriving qw_sb[d] unlocks 4 matmuls; d-order starts at
            # d=1 to buffer one in-flight tile; psS tile halves are
            # bank-aligned so per-group start=True is safe)
            dorder = [1, 0] + list(range(2, NDT))
            for wave in range(2):
                jts = [4 * wave + i for i in range(4)]
                g01 = [psV.tile([128, P], f32, name=f"ps_q{jt}", tag="psV")
                       for jt in jts[:2]]
                gs = psS.tile([128, 1024], f32, name=f"ps_q{wave}s",
                              tag="psS")
                groups = [g01[0], g01[1], gs[:, 0:512], gs[:, 512:1024]]
                for di, d in enumerate(dorder if wave == 0 else range(NDT)):
                    for i, jt in enumerate(jts):
                        nc.tensor.matmul(
                            groups[i],
                            lhsT=qw_sb[d][:, jt * 128:(jt + 1) * 128],
                            rhs=ts_d(d), start=(di == 0),
                            stop=(di == NDT - 1))
                for i, jt in enumerate(jts):
                    qt = persist.tile([128, P], bf16, name=f"qt_sb{jt}",
                                      tag=f"qt_sb{jt}")
                    nc.vector.tensor_scalar_add(qt, groups[i],
                                                qkb_sb[:, jt:jt + 1])
                    qt_sb[jt] = qt

            # ======== KT wave 1: 6 d-major groups gated on llm stream ====
            # (bias adds ride the idle ACT engine so kt0 is ready with no
            # DVE serialization on the critical path to head 0)
            kt_sb[0] = persist.tile([128, S], bf16, name="kt_sb0",
                                    tag="kt_sb0")
            kt_sb[1] = persist.tile([128, S], bf16, name="kt_sb1",
                                    tag="kt_sb1")
            w1 = [(0, 0), (0, 1), (0, 2), (0, 3), (1, 0), (1, 1)]
            gv = [psV.tile([128, 512], f32, name=f"ps_kw1_{i}", tag="psV")
                  for i in range(2)]
            gsa = psS.tile([128, 1024], f32, name="ps_kw1a", tag="psS")
            gsb = psS.tile([128, 1024], f32, name="ps_kw1b", tag="psS")
            kgroups = [gv[0], gv[1], gsa[:, 0:512], gsa[:, 512:1024],
                       gsb[:, 0:512], gsb[:, 512:1024]]
            for di, d in enumerate(dorder):
                for i, (jt, sc) in enumerate(w1):
                    nc.tensor.matmul(
                        kgroups[i],
                        lhsT=kw_sb[d][:, jt * 128:(jt + 1) * 128],
                        rhs=llm_sb[d][:, sc * 512:(sc + 1) * 512],
                        start=(di == 0), stop=(di == NDT - 1))
            for i, (jt, sc) in enumerate(w1):
                nc.scalar.activation(
                    kt_sb[jt][:, sc * 512:(sc + 1) * 512], kgroups[i],
                    Ident, bias=qkb_sb[:, NDT + jt:NDT + jt + 1])

            def emit_kt_chunk(jt, scs):
                if kt_sb[jt] is None:
                    kt_sb[jt] = persist.tile([128, S], bf16,
                                             name=f"kt_sb{jt}",
                                             tag=f"kt_sb{jt}")
                for sc in scs:
                    ps = psV.tile([128, 512], f32, name=f"ps_k{jt}_{sc}",
                                  tag="psV")
                    for d in range(NDT):
                        nc.tensor.matmul(
                            ps, lhsT=kw_sb[d][:, jt * 128:(jt + 1) * 128],
                            rhs=llm_sb[d][:, sc * 512:(sc + 1) * 512],
                            start=(d == 0), stop=(d == NDT - 1))
                    nc.vector.tensor_scalar_add(
                        kt_sb[jt][:, sc * 512:(sc + 1) * 512], ps,
                        qkb_sb[:, NDT + jt:NDT + jt + 1])

            def emit_v(st):
                # V'[s, h*65 + x]: x<64 -> v_h columns, x=64 -> ones
                vp = persist.tile([128, H * (DH + 1)], bf16,
                                  name=f"vp_sb{st}", tag=f"vp_sb{st}")
                vp3 = vp.rearrange("p (h x) -> p h x", x=DH + 1)
                nc.vector.memset(vp3[:, :, DH:DH + 1], 1.0)
                for jc in range(2):
                    ps = psV.tile([128, 512], f32, name=f"ps_v{st}_{jc}",
                                  tag="psV")
                    for d in range(NDT):
                        nc.tensor.matmul(
                            ps, lhsT=llm_sb[d][:, st * 128:(st + 1) * 128],
                            rhs=vw_sb[d][:, jc * 512:(jc + 1) * 512],
                            start=(d == 0), stop=(d == NDT - 1))
                    nc.vector.tensor_add(
                        vp3[:, jc * 8:(jc + 1) * 8, 0:DH],
                        ps.rearrange("p (h x) -> p h x", x=DH),
                        vbb_sb[:, jc * 512:(jc + 1) * 512]
                        .rearrange("p (h x) -> p h x", x=DH))
                vp_sb[st] = vp

            emitted_v = [0]

            def ensure_v(upto):
                while emitted_v[0] <= upto:
                    emit_v(emitted_v[0])
                    emitted_v[0] += 1

            psc_of = [None] * H

            def emit_normalize(h, pts=range(NPT)):
                # per-partition reciprocal of denominator column, then
                # scale the 64 ctx columns into ctx_pd
                psc = psc_of[h]
                for pt in pts:
                    c0 = pt * (DH + 1)
                    rc = rpool.tile([128, 1], f32, name=f"rc{h}_{pt}",
                                    tag="rc")
                    nc.vector.reciprocal(rc, psc[:, c0 + DH:c0 + DH + 1])
                    nc.vector.tensor_scalar_mul(
                        ctx_pd[pt][:, h * DH:(h + 1) * DH],
                        psc[:, c0:c0 + DH], rc)

            def emit_transposes(jt, pts=range(NPT)):
                # [p,d] -> [d,p] on the DMA xbar, off the PE critical path
                for pt in pts:
                    nc.sync.dma_start_transpose(
                        out=cx_sb[jt][:, pt * 128:(pt + 1) * 128],
                        in_=ctx_pd[pt][:, jt * 128:(jt + 1) * 128])

            def oproj_p1(pt, jc):
                # o-proj partial: d-tiles 0..5 -> bf16 partial (+ o_b)
                ps = psV.tile([128, 512], f32, name=f"ps_p1_{pt}_{jc}",
                              tag="psV")
                for d in range(6):
                    nc.tensor.matmul(
                        ps, lhsT=cx_sb[d][:, pt * 128:(pt + 1) * 128],
                        rhs=ow_sb[d][:, jc * 512:(jc + 1) * 512],
                        start=(d == 0), stop=(d == 5))
                nc.vector.tensor_add(otp[(pt, jc)], ps,
                                     obb_sb[:, jc * 512:(jc + 1) * 512])

            # ======================= heads =======================
            for h in range(H):
                jt = h // 2
                rs = (h % 2) * DH
                if h >= 1:
                    emit_normalize(h - 1)
                    if h % 2 == 0:
                        emit_transposes(jt - 1)
                # ctx accumulator: 4 p-tiles x (64 ctx cols + denom col).
                # All 4 groups share one PSUM bank, and a matmul with
                # start=True zeroes the whole bank -- so zero the tile once
                # and accumulate with start=False in every matmul.
                psc = psC.tile([128, NPT * (DH + 1)], f32, name=f"ps_c{h}",
                               tag="psC")
                nc.vector.memset(psc, 0.0)
                psc_of[h] = psc
                p1 = list(OP1.get(h, ()))
                for g in range(NG):
                    pss = psS.tile([128, 1024], f32, name=f"ps_s{h}_{g}",
                                   tag="psS")
                    for i in range(2):
                        st = 2 * g + i
                        nc.tensor.matmul(
                            pss[:, i * 512:(i + 1) * 512],
                            lhsT=kt_sb[jt][rs:rs + DH,
                                           st * 128:(st + 1) * 128],
                            rhs=qt_sb[jt][rs:rs + DH, :],
                            start=True, stop=True)
                    et = expool.tile([128, 1024], bf16, name=f"et{h}_{g}",
                                     tag="et")
                    nc.scalar.activation(et, pss, Exp, bias=0.0, scale=0.125)
                    # mid-head PE filler under the ACT-bound exp stream
                    if g == 0 and h in KT_CHUNKS:
                        emit_kt_chunk(*KT_CHUNKS[h])
                    if g in (2, 4, 6) and p1:
                        oproj_p1(*p1.pop(0))
                    ensure_v(2 * g + 1)
                    for i in range(2):
                        st = 2 * g + i
                        for pt in range(NPT):
                            nc.tensor.matmul(
                                psc[:, pt * (DH + 1):(pt + 1) * (DH + 1)],
                                lhsT=et[:, i * 512 + pt * 128:
                                        i * 512 + (pt + 1) * 128],
                                rhs=vp_sb[st][:, h * (DH + 1):
                                              (h + 1) * (DH + 1)],
                                start=False, stop=(st == NST - 1))
                while p1:
                    oproj_p1(*p1.pop(0))

            # ======== tail: last-head normalize + jt7 transpose (on PE,
            # short latency) + o-proj pass 2 (d-tiles 6,7 + partial) ====
            for pt in range(NPT):
                emit_normalize(H - 1, (pt,))
                pst = psC.tile([128, 128], bf16, name=f"pst{pt}", tag="psC")
                nc.tensor.transpose(
                    pst, ctx_pd[pt][:, (NDT - 1) * 128:NDT * 128], idm_sb)
                nc.vector.tensor_copy(
                    cx_sb[NDT - 1][:, pt * 128:(pt + 1) * 128], pst)
                for jc in range(2):
                    ps = psV.tile([128, 512], f32, name=f"ps_p2_{pt}_{jc}",
                                  tag="psV")
                    for d in (6, 7):
                        nc.tensor.matmul(
                            ps, lhsT=cx_sb[d][:, pt * 128:(pt + 1) * 128],
                            rhs=ow_sb[d][:, jc * 512:(jc + 1) * 512],
                            start=(d == 6), stop=(d == 7))
                    ot = opool.tile([128, 512], f32, name=f"ot{pt}_{jc}",
                                    tag="ot")
                    nc.vector.tensor_add(ot, ps, otp[(pt, jc)])
                    nc.sync.dma_start(
                        out=out.ap()[pt * 128:(pt + 1) * 128,
                                     jc * 512:(jc + 1) * 512],
                        in_=ot)


def get_nc():
    global _cached_nc
    if _cached_nc is None:
        _cached_nc = _build_nc()
    return _cached_nc


def make_in_maps(ts_features, llm_features, q_w, q_b, k_w, k_b, v_w, v_b,
                 o_w, o_b):
    ts = np.asarray(ts_features, np.float32)
    llm = np.asarray(llm_features, np.float32)
    shared = {
        "qwT": np.ascontiguousarray(np.asarray(q_w, np.float32).T).astype(_BF16),
        "kwT": np.ascontiguousarray(np.asarray(k_w, np.float32).T).astype(_BF16),
        "vwT": np.ascontiguousarray(np.asarray(v_w, np.float32).T).astype(_BF16),
        "owT": np.ascontiguousarray(np.asarray(o_w, np.float32).T).astype(_BF16),
        "qkb": np.ascontiguousarray(np.concatenate(
            [np.asarray(q_b, np.float32).reshape(NDT, 128).T,
             np.asarray(k_b, np.float32).reshape(NDT, 128).T], axis=1)),
        "idm": np.eye(128, dtype=np.float32).astype(_BF16),
        "vbb": np.ascontiguousarray(
            np.broadcast_to(np.asarray(v_b, np.float32), (128, D))).astype(_BF16),
        "obb": np.ascontiguousarray(
            np.broadcast_to(np.asarray(o_b, np.float32), (128, D))),
    }
    in_maps = []
    for b in range(NCORES):
        m = dict(shared)
        # ts2[r, d*512 + p] = ts[b].T[d*128 + r, p]
        m["ts2"] = np.ascontiguousarray(
            ts[b].T.reshape(NDT, 128, P).transpose(1, 0, 2)
            .reshape(128, NDT * P)).astype(_BF16)
        m["llmT"] = np.ascontiguousarray(llm[b].T).astype(_BF16)
        in_maps.append(m)
    return in_maps


def kernel(**inputs):
    from concourse.bass_utils import run_bass_kernel_spmd

    nc = get_nc()
    in_maps = make_in_maps(**inputs)
    res = run_bass_kernel_spmd(nc, in_maps, list(range(NCORES)))
    return np.stack([res.results[i]["out"] for i in range(NCORES)], axis=0)


# revision 32
# speedup vs baseline: 1.2303x; 1.0165x over previous
"""CrossAttention kernel for 8 Trainium2 NeuronCores.

Reference computation (per batch element b):
    q = ts[b] @ q_w.T + q_b          # [512, 1024]
    k = llm[b] @ k_w.T + k_b         # [2048, 1024]
    v = llm[b] @ v_w.T + v_b         # [2048, 1024]
    per head h (16 heads x 64 dims):
        scores = q_h @ k_h.T / 8     # [512, 2048]
        attn = softmax(scores, -1)
        ctx_h = attn @ v_h           # [512, 64]
    out = ctx @ o_w.T + o_b          # [512, 1024]

Sharding: data-parallel over batch (B=8 -> one element per core), no
collectives.

Per-core layout strategy (cost model: matmul time ~ out-free-size only,
so maximize output partitions and minimize re-streamed rows):

  QT[j, p]  = q_w @ ts.T  + q_b       (feature-major, bias per-partition)
  KT[j, s]  = k_w @ llm.T + k_b
  V'[s, j'] = llm @ v_w.T + v_b       (natural layout; j' = 16 heads x 65
                                       cols, col 64 of each head block is
                                       ones -> softmax denominator)
  scoresT_h[s, p] = KT_h.T @ QT_h     (K=64 contraction, N=512)
  expT = exp(scoresT / 8)             (no max subtraction: |scores/8| < ~3)
  ctx_h[p, 0:65] = expT_pt.T @ V'_h   (N=65 matmuls accumulated over s;
                                       col 64 = softmax denominator)
  normalize: rc = 1/ctx[:, 64];  ctx_pd[pt][:, h*64:+64] = ctx[:,0:64]*rc
                                      (per-partition scalar -> cheap DVE)
  cxT[jt] = transpose(ctx_pd)         (DMA xbar for jt<7, PE transpose +
                                       DVE copy for jt7 to cut latency)
  out[p, j] = cxT.T @ o_wT + o_b      (split: d-tiles 0..5 pre-accumulated
                                       into bf16 partials during the
                                       ACT-bound last heads; d 6..7 + add
                                       in a short tail)

Schedule: PE warms its p-state on dummy matmuls during the initial DMA
wait; QT runs d-major in two 4-group waves (one-tile arrival buffer) so
the q_w stream feeds it continuously; KT starts as a 6-group d-major
wave gated on the llm stream (bias adds on the idle ACT engine); the
remaining KT column chunks are spread one per head as mid-head PE
filler under the ACT-bound exp stream.

All matmuls bf16 inputs / fp32 PSUM accumulate.  Host does layout-only
prep (transpose, bf16 cast, bias broadcast, identity matrix).
"""
import numpy as np
import ml_dtypes

D = 1024          # d_model
P = 512           # ts sequence length
S = 2048          # llm sequence length
H = 16            # heads
DH = 64           # head dim
NCORES = 8
NDT = D // 128    # 8 d-tiles
NST = S // 128    # 16 s-tiles
NPT = P // 128    # 4 p-tiles
NG = NST // 2     # 8 score groups per head (2 s-tiles each)
NWARM = 100       # PE p-state warmup matmuls during initial DMA wait

# head -> (jt, [sc...]) remaining KT chunk emitted inside that head
KT_CHUNKS = {
    1: (2, (0, 1)), 2: (2, (2, 3)), 3: (3, (0, 1)),
    4: (3, (2, 3)), 5: (4, (0, 1)), 6: (4, (2, 3)), 7: (5, (0, 1)),
    8: (5, (2, 3)), 9: (6, (0, 1)), 10: (6, (2, 3)), 11: (7, (0, 1)),
    12: (7, (2, 3)),
}
# head -> o-proj (pt, jc, depth) groups whose d-tiles 0..depth-1 are
# pre-accumulated as mid-head filler (the last heads have no KT chunks
# left); depth is bounded by which cx_sb transposes have landed
OP1 = {12: ((0, 0, 6),), 13: ((0, 1, 6), (1, 0, 6), (1, 1, 6)),
       14: ((2, 0, 6), (2, 1, 6)), 15: ((3, 0, 6), (3, 1, 6))}
OP2 = {(0, 0): 6, (0, 1): 6, (1, 0): 6, (1, 1): 6,
       (2, 0): 6, (2, 1): 6, (3, 0): 6, (3, 1): 6}

_BF16 = ml_dtypes.bfloat16

_cached_nc = None


def _build_nc():
    import concourse.tile as tile
    from concourse import bacc, mybir

    f32 = mybir.dt.float32
    bf16 = mybir.dt.bfloat16
    Exp = mybir.ActivationFunctionType.Exp
    Ident = mybir.ActivationFunctionType.Identity

    nc = bacc.Bacc("TRN2", target_bir_lowering=False, debug=False,
                   num_devices=NCORES)

    ts2 = nc.declare_dram_parameter("ts2", [128, NDT * P], bf16,
                                    isOutput=False)
    llmT = nc.declare_dram_parameter("llmT", [D, S], bf16, isOutput=False)
    qwT = nc.declare_dram_parameter("qwT", [D, D], bf16, isOutput=False)
    kwT = nc.declare_dram_parameter("kwT", [D, D], bf16, isOutput=False)
    vwT = nc.declare_dram_parameter("vwT", [D, D], bf16, isOutput=False)
    owT = nc.declare_dram_parameter("owT", [D, D], bf16, isOutput=False)
    qkb = nc.declare_dram_parameter("qkb", [128, 2 * NDT], f32, isOutput=False)
    idm = nc.declare_dram_parameter("idm", [128, 128], bf16, isOutput=False)
    vbb = nc.declare_dram_parameter("vbb", [128, D], bf16, isOutput=False)
    obb = nc.declare_dram_parameter("obb", [128, D], f32, isOutput=False)
    out = nc.declare_dram_parameter("out", [P, D], bf16, isOutput=True)

    with tile.TileContext(nc) as tc:
        _emit(tc, nc, tile, mybir, f32, bf16, Exp, Ident,
              ts2, llmT, qwT, kwT, vwT, owT, qkb, idm, vbb, obb, out)
    nc.compile()
    return nc


def _emit(tc, nc, tile, mybir, f32, bf16, Exp, Ident,
          ts2, llmT, qwT, kwT, vwT, owT, qkb, idm, vbb, obb, out):
    from contextlib import ExitStack

    with ExitStack() as ctx:
        persist = ctx.enter_context(tc.tile_pool(name="persist", bufs=1))
        wpool = ctx.enter_context(tc.tile_pool(name="wpool", bufs=20))
        expool = ctx.enter_context(tc.tile_pool(name="expool", bufs=6))
        rpool = ctx.enter_context(tc.tile_pool(name="rpool", bufs=4))
        opool = ctx.enter_context(tc.tile_pool(name="opool", bufs=3))

        # ---- DMAs in first-use order ----
        qkb_sb = persist.tile([128, 2 * NDT], f32, name="qkb_sb", tag="qkb_sb")
        nc.sync.dma_start(out=qkb_sb, in_=qkb.ap())
        idm_sb = persist.tile([128, 128], bf16, name="idm_sb", tag="idm_sb")

        def load_w(dram, prefix):
            tiles = []
            for d in range(NDT):
                t = wpool.tile([128, D], bf16, name=f"{prefix}{d}", tag="w")
                nc.sync.dma_start(out=t, in_=dram.ap()[d * 128:(d + 1) * 128, :])
                tiles.append(t)
            return tiles

        # ts in two halves (batched: 8 small DMAs -> 2), interleaved with
        # the q_w tiles that the d-major QT waves consume in order.
        ts_sb = []
        for half in range(2):
            t = persist.tile([128, 4 * P], bf16, name=f"ts_sb{half}",
                             tag=f"ts_sb{half}")
            ts_sb.append(t)
        qw_sb = []
        nc.sync.dma_start(out=ts_sb[0], in_=ts2.ap()[:, 0:4 * P])
        for d in range(4):
            t = wpool.tile([128, D], bf16, name=f"qw_sb{d}", tag="w")
            nc.sync.dma_start(out=t, in_=qwT.ap()[d * 128:(d + 1) * 128, :])
            qw_sb.append(t)
        nc.sync.dma_start(out=ts_sb[1], in_=ts2.ap()[:, 4 * P:8 * P])
        for d in range(4, NDT):
            t = wpool.tile([128, D], bf16, name=f"qw_sb{d}", tag="w")
            nc.sync.dma_start(out=t, in_=qwT.ap()[d * 128:(d + 1) * 128, :])
            qw_sb.append(t)

        kw_sb = load_w(kwT, "kw_sb")
        llm_sb = []
        for d in range(NDT):
            t = persist.tile([128, S], bf16, name=f"llm_sb{d}", tag=f"llm_sb{d}")
            nc.sync.dma_start(out=t, in_=llmT.ap()[d * 128:(d + 1) * 128, :])
            llm_sb.append(t)
        vw_sb = load_w(vwT, "vw_sb")
        vbb_sb = persist.tile([128, D], bf16, name="vbb_sb", tag="vbb_sb")
        nc.sync.dma_start(out=vbb_sb, in_=vbb.ap())
        ow_sb = load_w(owT, "ow_sb")
        obb_sb = persist.tile([128, D], f32, name="obb_sb", tag="obb_sb")
        nc.sync.dma_start(out=obb_sb, in_=obb.ap())
        nc.sync.dma_start(out=idm_sb, in_=idm.ap())

        def ts_d(d):
            return ts_sb[d // 4][:, (d % 4) * P:(d % 4 + 1) * P]

        # ---- persistent activation/result tiles ----
        qt_sb = [None] * NDT
        kt_sb = [None] * NDT
        vp_sb = [None] * NST
        cx_sb = [persist.tile([128, P], bf16, name=f"cx_sb{jt}",
                              tag=f"cx_sb{jt}") for jt in range(NDT)]
        ctx_pd = [persist.tile([128, D], bf16, name=f"ctx_pd{pt}",
                               tag=f"ctx_pd{pt}") for pt in range(NPT)]
        # bf16 o-proj partials (d-tiles 0..5 + o_b)
        otp = {(pt, jc): persist.tile([128, 512], bf16,
                                      name=f"otp{pt}_{jc}",
                                      tag=f"otp{pt}_{jc}")
               for pt in range(NPT) for jc in range(2)}

        with tc.tile_pool(name="psS", bufs=2, space="PSUM") as psS, \
             tc.tile_pool(name="psC", bufs=2, space="PSUM") as psC, \
             tc.tile_pool(name="psV", bufs=2, space="PSUM") as psV:

            # ---- PE p-state warmup on qkb during the initial DMA wait ----
            wps = psV.tile([128, 16], f32, name="ps_warm", tag="psV")
            for i in range(NWARM):
                nc.tensor.matmul(wps[0:16, :], lhsT=qkb_sb[:, 0:16],
                                 rhs=qkb_sb, start=True, stop=True)

            # ======== QT: d-major, two waves of 4 concurrent j-groups ====
            # (each arriving qw_sb[d] unlocks 4 matmuls; d-order starts at
            # d=1 to buffer one in-flight tile; psS tile halves are
            # bank-aligned so per-group start=True is safe)
            dorder = [1, 0] + list(range(2, NDT))
            for wave in range(2):
                jts = [4 * wave + i for i in range(4)]
                g01 = [psV.tile([128, P], f32, name=f"ps_q{jt}", tag="psV")
                       for jt in jts[:2]]
                gs = psS.tile([128, 1024], f32, name=f"ps_q{wave}s",
                              tag="psS")
                groups = [g01[0], g01[1], gs[:, 0:512], gs[:, 512:1024]]
                for di, d in enumerate(dorder if wave == 0 else range(NDT)):
                    for i, jt in enumerate(jts):
                        nc.tensor.matmul(
                            groups[i],
                            lhsT=qw_sb[d][:, jt * 128:(jt + 1) * 128],
                            rhs=ts_d(d), start=(di == 0),
                            stop=(di == NDT - 1))
                for i, jt in enumerate(jts):
                    qt = persist.tile([128, P], bf16, name=f"qt_sb{jt}",
                                      tag=f"qt_sb{jt}")
                    nc.vector.tensor_scalar_add(qt, groups[i],
                                                qkb_sb[:, jt:jt + 1])
                    qt_sb[jt] = qt

            # ======== KT wave 1: 6 d-major groups gated on llm stream ====
            # (bias adds ride the idle ACT engine so kt0 is ready with no
            # DVE serialization on the critical path to head 0)
            kt_sb[0] = persist.tile([128, S], bf16, name="kt_sb0",
                                    tag="kt_sb0")
            kt_sb[1] = persist.tile([128, S], bf16, name="kt_sb1",
                                    tag="kt_sb1")
            w1 = [(0, 0), (0, 1), (0, 2), (0, 3),
                  (1, 0), (1, 1), (1, 2), (1, 3)]
            gv = [psV.tile([128, 512], f32, name=f"ps_kw1_{i}", tag="psV")
                  for i in range(2)]
            gc = [psC.tile([128, 512], f32, name=f"ps_kw1c_{i}", tag="psC")
                  for i in range(2)]
            gsa = psS.tile([128, 1024], f32, name="ps_kw1a", tag="psS")
            gsb = psS.tile([128, 1024], f32, name="ps_kw1b", tag="psS")
            kgroups = [gv[0], gv[1], gsa[:, 0:512], gsa[:, 512:1024],
                       gsb[:, 0:512], gsb[:, 512:1024], gc[0], gc[1]]
            for di, d in enumerate(dorder):
                for i, (jt, sc) in enumerate(w1):
                    nc.tensor.matmul(
                        kgroups[i],
                        lhsT=kw_sb[d][:, jt * 128:(jt + 1) * 128],
                        rhs=llm_sb[d][:, sc * 512:(sc + 1) * 512],
                        start=(di == 0), stop=(di == NDT - 1))
            for i, (jt, sc) in enumerate(w1):
                nc.scalar.activation(
                    kt_sb[jt][:, sc * 512:(sc + 1) * 512], kgroups[i],
                    Ident, bias=qkb_sb[:, NDT + jt:NDT + jt + 1])

            def emit_kt_chunk(jt, scs):
                if kt_sb[jt] is None:
                    kt_sb[jt] = persist.tile([128, S], bf16,
                                             name=f"kt_sb{jt}",
                                             tag=f"kt_sb{jt}")
                for sc in scs:
                    ps = psV.tile([128, 512], f32, name=f"ps_k{jt}_{sc}",
                                  tag="psV")
                    for d in range(NDT):
                        nc.tensor.matmul(
                            ps, lhsT=kw_sb[d][:, jt * 128:(jt + 1) * 128],
                            rhs=llm_sb[d][:, sc * 512:(sc + 1) * 512],
                            start=(d == 0), stop=(d == NDT - 1))
                    nc.vector.tensor_scalar_add(
                        kt_sb[jt][:, sc * 512:(sc + 1) * 512], ps,
                        qkb_sb[:, NDT + jt:NDT + jt + 1])

            def emit_v(st):
                # V'[s, h*65 + x]: x<64 -> v_h columns, x=64 -> ones
                vp = persist.tile([128, H * (DH + 1)], bf16,
                                  name=f"vp_sb{st}", tag=f"vp_sb{st}")
                vp3 = vp.rearrange("p (h x) -> p h x", x=DH + 1)
                nc.vector.memset(vp3[:, :, DH:DH + 1], 1.0)
                for jc in range(2):
                    ps = psV.tile([128, 512], f32, name=f"ps_v{st}_{jc}",
                                  tag="psV")
                    for d in range(NDT):
                        nc.tensor.matmul(
                            ps, lhsT=llm_sb[d][:, st * 128:(st + 1) * 128],
                            rhs=vw_sb[d][:, jc * 512:(jc + 1) * 512],
                            start=(d == 0), stop=(d == NDT - 1))
                    nc.vector.tensor_add(
                        vp3[:, jc * 8:(jc + 1) * 8, 0:DH],
                        ps.rearrange("p (h x) -> p h x", x=DH),
                        vbb_sb[:, jc * 512:(jc + 1) * 512]
                        .rearrange("p (h x) -> p h x", x=DH))
                vp_sb[st] = vp

            emitted_v = [0]

            def ensure_v(upto):
                while emitted_v[0] <= upto:
                    emit_v(emitted_v[0])
                    emitted_v[0] += 1

            psc_of = [None] * H

            def emit_normalize(h, pts=range(NPT)):
                # per-partition reciprocal of denominator column, then
                # scale the 64 ctx columns into ctx_pd
                psc = psc_of[h]
                for pt in pts:
                    c0 = pt * (DH + 1)
                    rc = rpool.tile([128, 1], f32, name=f"rc{h}_{pt}",
                                    tag="rc")
                    nc.vector.reciprocal(rc, psc[:, c0 + DH:c0 + DH + 1])
                    nc.vector.tensor_scalar_mul(
                        ctx_pd[pt][:, h * DH:(h + 1) * DH],
                        psc[:, c0:c0 + DH], rc)

            def emit_transposes(jt, pts=range(NPT)):
                # [p,d] -> [d,p] on the DMA xbar, off the PE critical path
                for pt in pts:
                    nc.sync.dma_start_transpose(
                        out=cx_sb[jt][:, pt * 128:(pt + 1) * 128],
                        in_=ctx_pd[pt][:, jt * 128:(jt + 1) * 128])

            def oproj_p1(pt, jc, depth):
                # o-proj partial: d-tiles 0..depth-1 -> bf16 partial (+ o_b)
                ps = psV.tile([128, 512], f32, name=f"ps_p1_{pt}_{jc}",
                              tag="psV")
                for d in range(depth):
                    nc.tensor.matmul(
                        ps, lhsT=cx_sb[d][:, pt * 128:(pt + 1) * 128],
                        rhs=ow_sb[d][:, jc * 512:(jc + 1) * 512],
                        start=(d == 0), stop=(d == depth - 1))
                nc.vector.tensor_add(otp[(pt, jc)], ps,
                                     obb_sb[:, jc * 512:(jc + 1) * 512])

            def alloc_psc(h):
                # ctx accumulator: 4 p-tiles x (64 ctx cols + denom col).
                # All 4 groups share one PSUM bank, and a matmul with
                # start=True zeroes the whole bank -- so zero the tile once
                # and accumulate with start=False in every matmul.
                psc = psC.tile([128, NPT * (DH + 1)], f32, name=f"ps_c{h}",
                               tag="psC")
                nc.vector.memset(psc, 0.0)
                psc_of[h] = psc
                return psc

            def emit_scores(h, q, jt, rs):
                pss = psS.tile([128, 1024], f32, name=f"ps_s{h}_{q}",
                               tag="psS")
                for i in range(2):
                    st = 2 * q + i
                    nc.tensor.matmul(
                        pss[:, i * 512:(i + 1) * 512],
                        lhsT=kt_sb[jt][rs:rs + DH, st * 128:(st + 1) * 128],
                        rhs=qt_sb[jt][rs:rs + DH, :],
                        start=True, stop=True)
                et = expool.tile([128, 1024], bf16, name=f"et{h}_{q}",
                                 tag="et")
                nc.scalar.activation(et, pss, Exp, bias=0.0, scale=0.125)
                return et

            def emit_ctx(h, q, et, psc):
                for i in range(2):
                    st = 2 * q + i
                    for pt in range(NPT):
                        nc.tensor.matmul(
                            psc[:, pt * (DH + 1):(pt + 1) * (DH + 1)],
                            lhsT=et[:, i * 512 + pt * 128:
                                    i * 512 + (pt + 1) * 128],
                            rhs=vp_sb[st][:, h * (DH + 1):
                                          (h + 1) * (DH + 1)],
                            start=False, stop=(st == NST - 1))

            # ======================= heads =======================
            for h in range(H):
                jt = h // 2
                rs = (h % 2) * DH
                if h >= 1:
                    emit_normalize(h - 1)
                    if h % 2 == 0:
                        emit_transposes(jt - 1)
                psc = alloc_psc(h)
                p1 = list(OP1.get(h, ()))
                for q in range(NG):
                    et = emit_scores(h, q, jt, rs)
                    # mid-head PE filler under the ACT-bound exp stream
                    if q == 0 and h in KT_CHUNKS:
                        emit_kt_chunk(*KT_CHUNKS[h])
                    if q in (2, 4, 6) and p1:
                        oproj_p1(*p1.pop(0))
                    ensure_v(2 * q + 1)
                    emit_ctx(h, q, et, psc)
                while p1:
                    oproj_p1(*p1.pop(0))

            # ======== tail: last-head normalize + jt7 transpose (on PE,
            # short latency) + o-proj pass 2 (d-tiles 6,7 + partial) ====
            Copy = mybir.ActivationFunctionType.Copy
            psc15 = psc_of[H - 1]
            for pt in range(NPT):
                # normalize-mul and transpose-copy ride the (now idle) ACT
                # engine; only the reciprocal needs DVE
                c0 = pt * (DH + 1)
                rc = rpool.tile([128, 1], f32, name=f"rc15_{pt}", tag="rc")
                nc.vector.reciprocal(rc, psc15[:, c0 + DH:c0 + DH + 1])
                nc.scalar.activation(
                    ctx_pd[pt][:, (H - 1) * DH:H * DH],
                    psc15[:, c0:c0 + DH], Copy, scale=rc)
                pst = psC.tile([128, 128], bf16, name=f"pst{pt}", tag="psC")
                nc.tensor.transpose(
                    pst, ctx_pd[pt][:, (NDT - 1) * 128:NDT * 128], idm_sb)
                nc.scalar.activation(
                    cx_sb[NDT - 1][:, pt * 128:(pt + 1) * 128], pst, Copy)
                ot = opool.tile([128, 1024], bf16, name=f"ot{pt}", tag="ot")
                for jc in range(2):
                    ps = psV.tile([128, 512], f32, name=f"ps_p2_{pt}_{jc}",
                                  tag="psV")
                    d0 = OP2[(pt, jc)]
                    for d in range(d0, NDT):
                        nc.tensor.matmul(
                            ps, lhsT=cx_sb[d][:, pt * 128:(pt + 1) * 128],
                            rhs=ow_sb[d][:, jc * 512:(jc + 1) * 512],
                            start=(d == d0), stop=(d == NDT - 1))
                    nc.vector.tensor_add(ot[:, jc * 512:(jc + 1) * 512],
                                         ps, otp[(pt, jc)])
                    if pt >= 2:
                        # split the last p-tiles' writeback per half so the
                        # final DMA is not gated on both merge adds
                        nc.sync.dma_start(
                            out=out.ap()[pt * 128:(pt + 1) * 128,
                                         jc * 512:(jc + 1) * 512],
                            in_=ot[:, jc * 512:(jc + 1) * 512])
                if pt < 2:
                    nc.sync.dma_start(
                        out=out.ap()[pt * 128:(pt + 1) * 128, :], in_=ot)


def get_nc():
    global _cached_nc
    if _cached_nc is None:
        _cached_nc = _build_nc()
    return _cached_nc


def make_in_maps(ts_features, llm_features, q_w, q_b, k_w, k_b, v_w, v_b,
                 o_w, o_b):
    ts = np.asarray(ts_features, np.float32)
    llm = np.asarray(llm_features, np.float32)
    shared = {
        "qwT": np.ascontiguousarray(np.asarray(q_w, np.float32).T).astype(_BF16),
        "kwT": np.ascontiguousarray(np.asarray(k_w, np.float32).T).astype(_BF16),
        "vwT": np.ascontiguousarray(np.asarray(v_w, np.float32).T).astype(_BF16),
        "owT": np.ascontiguousarray(np.asarray(o_w, np.float32).T).astype(_BF16),
        "qkb": np.ascontiguousarray(np.concatenate(
            [np.asarray(q_b, np.float32).reshape(NDT, 128).T,
             np.asarray(k_b, np.float32).reshape(NDT, 128).T], axis=1)),
        "idm": np.eye(128, dtype=np.float32).astype(_BF16),
        "vbb": np.ascontiguousarray(
            np.broadcast_to(np.asarray(v_b, np.float32), (128, D))).astype(_BF16),
        "obb": np.ascontiguousarray(
            np.broadcast_to(np.asarray(o_b, np.float32), (128, D))),
    }
    in_maps = []
    for b in range(NCORES):
        m = dict(shared)
        # ts2[r, d*512 + p] = ts[b].T[d*128 + r, p]
        m["ts2"] = np.ascontiguousarray(
            ts[b].T.reshape(NDT, 128, P).transpose(1, 0, 2)
            .reshape(128, NDT * P)).astype(_BF16)
        m["llmT"] = np.ascontiguousarray(llm[b].T).astype(_BF16)
        in_maps.append(m)
    return in_maps


def kernel(**inputs):
    from concourse.bass_utils import run_bass_kernel_spmd

    nc = get_nc()
    in_maps = make_in_maps(**inputs)
    res = run_bass_kernel_spmd(nc, in_maps, list(range(NCORES)))
    return np.stack([res.results[i]["out"] for i in range(NCORES)],
                    axis=0).astype(np.float32)


# revision 43
# speedup vs baseline: 1.2337x; 1.0028x over previous
"""CrossAttention kernel for 8 Trainium2 NeuronCores.

Reference computation (per batch element b):
    q = ts[b] @ q_w.T + q_b          # [512, 1024]
    k = llm[b] @ k_w.T + k_b         # [2048, 1024]
    v = llm[b] @ v_w.T + v_b         # [2048, 1024]
    per head h (16 heads x 64 dims):
        scores = q_h @ k_h.T / 8     # [512, 2048]
        attn = softmax(scores, -1)
        ctx_h = attn @ v_h           # [512, 64]
    out = ctx @ o_w.T + o_b          # [512, 1024]

Sharding: data-parallel over batch (B=8 -> one element per core), no
collectives.

Per-core layout strategy (cost model: matmul time ~ out-free-size only,
so maximize output partitions and minimize re-streamed rows):

  QT[j, p]  = q_w @ ts.T  + q_b       (feature-major, bias per-partition)
  KT[j, s]  = k_w @ llm.T + k_b
  V'[s, j'] = llm @ v_w.T + v_b       (natural layout; j' = 16 heads x 65
                                       cols, col 64 of each head block is
                                       ones -> softmax denominator)
  scoresT_h[s, p] = KT_h.T @ QT_h     (K=64 contraction, N=512)
  expT = exp(scoresT / 8)             (no max subtraction: |scores/8| < ~3)
  ctx_h[p, 0:65] = expT_pt.T @ V'_h   (N=65 matmuls accumulated over s;
                                       col 64 = softmax denominator)
  normalize: rc = 1/ctx[:, 64];  ctx_pd[pt][:, h*64:+64] = ctx[:,0:64]*rc
                                      (per-partition scalar -> cheap DVE)
  cxT[jt] = transpose(ctx_pd)         (DMA xbar for jt<7, PE transpose +
                                       DVE copy for jt7 to cut latency)
  out[p, j] = cxT.T @ o_wT + o_b      (split: d-tiles 0..5 pre-accumulated
                                       into bf16 partials during the
                                       ACT-bound last heads; d 6..7 + add
                                       in a short tail)

Schedule: PE warms its p-state on dummy matmuls during the initial DMA
wait; QT runs d-major in two 4-group waves (one-tile arrival buffer) so
the q_w stream feeds it continuously; KT starts as a 6-group d-major
wave gated on the llm stream (bias adds on the idle ACT engine); the
remaining KT column chunks are spread one per head as mid-head PE
filler under the ACT-bound exp stream.

All matmuls bf16 inputs / fp32 PSUM accumulate.  Host does layout-only
prep (transpose, bf16 cast, bias broadcast, identity matrix).
"""
import numpy as np
import ml_dtypes

D = 1024          # d_model
P = 512           # ts sequence length
S = 2048          # llm sequence length
H = 16            # heads
DH = 64           # head dim
NCORES = 8
NDT = D // 128    # 8 d-tiles
NST = S // 128    # 16 s-tiles
NPT = P // 128    # 4 p-tiles
NG = NST // 2     # 8 score groups per head (2 s-tiles each)
NWARM = 100       # PE p-state warmup matmuls during initial DMA wait

# head -> (jt, [sc...]) remaining KT chunk emitted inside that head
KT_CHUNKS = {
    1: (2, (0, 1)), 2: (2, (2, 3)), 3: (3, (0, 1)),
    4: (3, (2, 3)), 5: (4, (0, 1)), 6: (4, (2, 3)), 7: (5, (0, 1)),
    8: (5, (2, 3)), 9: (6, (0, 1)), 10: (6, (2, 3)), 11: (7, (0, 1)),
    12: (7, (2, 3)),
}
# head -> o-proj (pt, jc, depth) groups whose d-tiles 0..depth-1 are
# pre-accumulated as mid-head filler (the last heads have no KT chunks
# left); depth is bounded by which cx_sb transposes have landed
OP1 = {12: ((0, 0, 6),), 13: ((0, 1, 6), (1, 0, 6), (1, 1, 6)),
       14: ((2, 0, 6), (2, 1, 6)), 15: ((3, 0, 7), (3, 1, 7))}
OP2 = {(0, 0): 6, (0, 1): 6, (1, 0): 6, (1, 1): 6,
       (2, 0): 6, (2, 1): 6, (3, 0): 7, (3, 1): 7}

_BF16 = ml_dtypes.bfloat16

_cached_nc = None


def _build_nc():
    import concourse.tile as tile
    from concourse import bacc, mybir

    f32 = mybir.dt.float32
    bf16 = mybir.dt.bfloat16
    Exp = mybir.ActivationFunctionType.Exp
    Ident = mybir.ActivationFunctionType.Identity

    nc = bacc.Bacc("TRN2", target_bir_lowering=False, debug=False,
                   num_devices=NCORES)

    ts2 = nc.declare_dram_parameter("ts2", [128, NDT * P], bf16,
                                    isOutput=False)
    llmT = nc.declare_dram_parameter("llmT", [D, S], bf16, isOutput=False)
    qwT = nc.declare_dram_parameter("qwT", [D, D], bf16, isOutput=False)
    kwT = nc.declare_dram_parameter("kwT", [D, D], bf16, isOutput=False)
    vwT = nc.declare_dram_parameter("vwT", [D, D], bf16, isOutput=False)
    owT = nc.declare_dram_parameter("owT", [D, D], bf16, isOutput=False)
    qkb = nc.declare_dram_parameter("qkb", [128, 2 * NDT], f32, isOutput=False)
    idm = nc.declare_dram_parameter("idm", [128, 128], bf16, isOutput=False)
    vbb = nc.declare_dram_parameter("vbb", [128, D], bf16, isOutput=False)
    obb = nc.declare_dram_parameter("obb", [128, D], f32, isOutput=False)
    out = nc.declare_dram_parameter("out", [P, D], bf16, isOutput=True)

    with tile.TileContext(nc) as tc:
        _emit(tc, nc, tile, mybir, f32, bf16, Exp, Ident,
              ts2, llmT, qwT, kwT, vwT, owT, qkb, idm, vbb, obb, out)
    nc.compile()
    return nc


def _emit(tc, nc, tile, mybir, f32, bf16, Exp, Ident,
          ts2, llmT, qwT, kwT, vwT, owT, qkb, idm, vbb, obb, out):
    from contextlib import ExitStack

    with ExitStack() as ctx:
        persist = ctx.enter_context(tc.tile_pool(name="persist", bufs=1))
        wpool = ctx.enter_context(tc.tile_pool(name="wpool", bufs=20))
        expool = ctx.enter_context(tc.tile_pool(name="expool", bufs=6))
        rpool = ctx.enter_context(tc.tile_pool(name="rpool", bufs=4))
        opool = ctx.enter_context(tc.tile_pool(name="opool", bufs=3))

        # ---- DMAs in first-use order ----
        qkb_sb = persist.tile([128, 2 * NDT], f32, name="qkb_sb", tag="qkb_sb")
        nc.sync.dma_start(out=qkb_sb, in_=qkb.ap())
        idm_sb = persist.tile([128, 128], bf16, name="idm_sb", tag="idm_sb")

        def load_w(dram, prefix):
            tiles = []
            for d in range(NDT):
                t = wpool.tile([128, D], bf16, name=f"{prefix}{d}", tag="w")
                nc.sync.dma_start(out=t, in_=dram.ap()[d * 128:(d + 1) * 128, :])
                tiles.append(t)
            return tiles

        # ts in two halves (batched: 8 small DMAs -> 2), interleaved with
        # the q_w tiles that the d-major QT waves consume in order.
        ts_sb = []
        for half in range(2):
            t = persist.tile([128, 4 * P], bf16, name=f"ts_sb{half}",
                             tag=f"ts_sb{half}")
            ts_sb.append(t)
        qw_sb = []
        nc.sync.dma_start(out=ts_sb[0], in_=ts2.ap()[:, 0:4 * P])
        for d in range(4):
            t = wpool.tile([128, D], bf16, name=f"qw_sb{d}", tag="w")
            nc.sync.dma_start(out=t, in_=qwT.ap()[d * 128:(d + 1) * 128, :])
            qw_sb.append(t)
        nc.sync.dma_start(out=ts_sb[1], in_=ts2.ap()[:, 4 * P:8 * P])
        for d in range(4, NDT):
            t = wpool.tile([128, D], bf16, name=f"qw_sb{d}", tag="w")
            nc.sync.dma_start(out=t, in_=qwT.ap()[d * 128:(d + 1) * 128, :])
            qw_sb.append(t)

        kw_sb = load_w(kwT, "kw_sb")
        llm_sb = []
        for d in range(NDT):
            t = persist.tile([128, S], bf16, name=f"llm_sb{d}", tag=f"llm_sb{d}")
            nc.sync.dma_start(out=t, in_=llmT.ap()[d * 128:(d + 1) * 128, :])
            llm_sb.append(t)
        vw_sb = load_w(vwT, "vw_sb")
        vbb_sb = persist.tile([128, D], bf16, name="vbb_sb", tag="vbb_sb")
        nc.sync.dma_start(out=vbb_sb, in_=vbb.ap())
        ow_sb = load_w(owT, "ow_sb")
        obb_sb = persist.tile([128, D], f32, name="obb_sb", tag="obb_sb")
        nc.sync.dma_start(out=obb_sb, in_=obb.ap())
        nc.sync.dma_start(out=idm_sb, in_=idm.ap())

        def ts_d(d):
            return ts_sb[d // 4][:, (d % 4) * P:(d % 4 + 1) * P]

        # ---- persistent activation/result tiles ----
        qt_sb = [None] * NDT
        kt_sb = [None] * NDT
        vp_sb = [None] * NST
        cx_sb = [persist.tile([128, P], bf16, name=f"cx_sb{jt}",
                              tag=f"cx_sb{jt}") for jt in range(NDT)]
        ctx_pd = [persist.tile([128, D], bf16, name=f"ctx_pd{pt}",
                               tag=f"ctx_pd{pt}") for pt in range(NPT)]
        # bf16 o-proj partials (d-tiles 0..5 + o_b)
        otp = {(pt, jc): persist.tile([128, 512], bf16,
                                      name=f"otp{pt}_{jc}",
                                      tag=f"otp{pt}_{jc}")
               for pt in range(NPT) for jc in range(2)}

        with tc.tile_pool(name="psS", bufs=2, space="PSUM") as psS, \
             tc.tile_pool(name="psC", bufs=2, space="PSUM") as psC, \
             tc.tile_pool(name="psV", bufs=2, space="PSUM") as psV:

            # ---- PE p-state warmup on qkb during the initial DMA wait ----
            wps = psV.tile([128, 16], f32, name="ps_warm", tag="psV")
            for i in range(NWARM):
                nc.tensor.matmul(wps[0:16, :], lhsT=qkb_sb[:, 0:16],
                                 rhs=qkb_sb, start=True, stop=True)

            # ======== QT: d-major, two waves of 4 concurrent j-groups ====
            # (each arriving qw_sb[d] unlocks 4 matmuls; d-order starts at
            # d=1 to buffer one in-flight tile; psS tile halves are
            # bank-aligned so per-group start=True is safe)
            dorder = [1, 0] + list(range(2, NDT))
            for wave in range(2):
                jts = [4 * wave + i for i in range(4)]
                g01 = [psV.tile([128, P], f32, name=f"ps_q{jt}", tag="psV")
                       for jt in jts[:2]]
                gs = psS.tile([128, 1024], f32, name=f"ps_q{wave}s",
                              tag="psS")
                groups = [g01[0], g01[1], gs[:, 0:512], gs[:, 512:1024]]
                for di, d in enumerate(dorder if wave == 0 else range(NDT)):
                    for i, jt in enumerate(jts):
                        nc.tensor.matmul(
                            groups[i],
                            lhsT=qw_sb[d][:, jt * 128:(jt + 1) * 128],
                            rhs=ts_d(d), start=(di == 0),
                            stop=(di == NDT - 1))
                for i, jt in enumerate(jts):
                    qt = persist.tile([128, P], bf16, name=f"qt_sb{jt}",
                                      tag=f"qt_sb{jt}")
                    nc.vector.tensor_scalar_add(qt, groups[i],
                                                qkb_sb[:, jt:jt + 1])
                    qt_sb[jt] = qt

            # ======== KT wave 1: 6 d-major groups gated on llm stream ====
            # (bias adds ride the idle ACT engine so kt0 is ready with no
            # DVE serialization on the critical path to head 0)
            kt_sb[0] = persist.tile([128, S], bf16, name="kt_sb0",
                                    tag="kt_sb0")
            kt_sb[1] = persist.tile([128, S], bf16, name="kt_sb1",
                                    tag="kt_sb1")
            w1 = [(0, 0), (0, 1), (0, 2), (0, 3),
                  (1, 0), (1, 1), (1, 2), (1, 3)]
            gv = [psV.tile([128, 512], f32, name=f"ps_kw1_{i}", tag="psV")
                  for i in range(2)]
            gc = [psC.tile([128, 512], f32, name=f"ps_kw1c_{i}", tag="psC")
                  for i in range(2)]
            gsa = psS.tile([128, 1024], f32, name="ps_kw1a", tag="psS")
            gsb = psS.tile([128, 1024], f32, name="ps_kw1b", tag="psS")
            kgroups = [gv[0], gv[1], gsa[:, 0:512], gsa[:, 512:1024],
                       gsb[:, 0:512], gsb[:, 512:1024], gc[0], gc[1]]
            for di, d in enumerate(dorder):
                for i, (jt, sc) in enumerate(w1):
                    nc.tensor.matmul(
                        kgroups[i],
                        lhsT=kw_sb[d][:, jt * 128:(jt + 1) * 128],
                        rhs=llm_sb[d][:, sc * 512:(sc + 1) * 512],
                        start=(di == 0), stop=(di == NDT - 1))
            for i, (jt, sc) in enumerate(w1):
                nc.scalar.activation(
                    kt_sb[jt][:, sc * 512:(sc + 1) * 512], kgroups[i],
                    Ident, bias=qkb_sb[:, NDT + jt:NDT + jt + 1])

            def emit_kt_sc(jt, sc):
                if kt_sb[jt] is None:
                    kt_sb[jt] = persist.tile([128, S], bf16,
                                             name=f"kt_sb{jt}",
                                             tag=f"kt_sb{jt}")
                ps = psV.tile([128, 512], f32, name=f"ps_k{jt}_{sc}",
                              tag="psV")
                for d in range(NDT):
                    nc.tensor.matmul(
                        ps, lhsT=kw_sb[d][:, jt * 128:(jt + 1) * 128],
                        rhs=llm_sb[d][:, sc * 512:(sc + 1) * 512],
                        start=(d == 0), stop=(d == NDT - 1))
                nc.vector.tensor_scalar_add(
                    kt_sb[jt][:, sc * 512:(sc + 1) * 512], ps,
                    qkb_sb[:, NDT + jt:NDT + jt + 1])

            def emit_kt_chunk(jt, scs):
                for sc in scs:
                    emit_kt_sc(jt, sc)

            def emit_v(st):
                # V'[s, h*65 + x]: x<64 -> v_h columns, x=64 -> ones
                vp = persist.tile([128, H * (DH + 1)], bf16,
                                  name=f"vp_sb{st}", tag=f"vp_sb{st}")
                vp3 = vp.rearrange("p (h x) -> p h x", x=DH + 1)
                nc.vector.memset(vp3[:, :, DH:DH + 1], 1.0)
                for jc in range(2):
                    ps = psV.tile([128, 512], f32, name=f"ps_v{st}_{jc}",
                                  tag="psV")
                    for d in range(NDT):
                        nc.tensor.matmul(
                            ps, lhsT=llm_sb[d][:, st * 128:(st + 1) * 128],
                            rhs=vw_sb[d][:, jc * 512:(jc + 1) * 512],
                            start=(d == 0), stop=(d == NDT - 1))
                    nc.vector.tensor_add(
                        vp3[:, jc * 8:(jc + 1) * 8, 0:DH],
                        ps.rearrange("p (h x) -> p h x", x=DH),
                        vbb_sb[:, jc * 512:(jc + 1) * 512]
                        .rearrange("p (h x) -> p h x", x=DH))
                vp_sb[st] = vp

            emitted_v = [0]

            def ensure_v(upto):
                while emitted_v[0] <= upto:
                    emit_v(emitted_v[0])
                    emitted_v[0] += 1

            psc_of = [None] * H

            def emit_normalize(h, pts=range(NPT)):
                # per-partition reciprocal of denominator column, then
                # scale the 64 ctx columns into ctx_pd
                psc = psc_of[h]
                for pt in pts:
                    c0 = pt * (DH + 1)
                    rc = rpool.tile([128, 1], f32, name=f"rc{h}_{pt}",
                                    tag="rc")
                    nc.vector.reciprocal(rc, psc[:, c0 + DH:c0 + DH + 1])
                    nc.vector.tensor_scalar_mul(
                        ctx_pd[pt][:, h * DH:(h + 1) * DH],
                        psc[:, c0:c0 + DH], rc)

            def emit_transposes(jt, pts=range(NPT)):
                # [p,d] -> [d,p] on the DMA xbar, off the PE critical path
                for pt in pts:
                    nc.sync.dma_start_transpose(
                        out=cx_sb[jt][:, pt * 128:(pt + 1) * 128],
                        in_=ctx_pd[pt][:, jt * 128:(jt + 1) * 128])

            def oproj_p1_tasks(pt, jc, depth):
                # o-proj partial: d-tiles 0..depth-1 -> bf16 partial (+ o_b)
                # split into ~3-matmul microtasks so the filler spreads
                # evenly across the head's exp-latency bubbles
                box = {}

                def start(ds):
                    def fn():
                        box['ps'] = psV.tile([128, 512], f32,
                                             name=f"ps_p1_{pt}_{jc}",
                                             tag="psV")
                        step(ds)()
                    return fn

                def step(ds):
                    def fn():
                        for d in ds:
                            nc.tensor.matmul(
                                box['ps'],
                                lhsT=cx_sb[d][:, pt * 128:(pt + 1) * 128],
                                rhs=ow_sb[d][:, jc * 512:(jc + 1) * 512],
                                start=(d == 0), stop=(d == depth - 1))
                        if ds[-1] == depth - 1:
                            nc.vector.tensor_add(
                                otp[(pt, jc)], box['ps'],
                                obb_sb[:, jc * 512:(jc + 1) * 512])
                    return fn

                ds = list(range(depth))
                return [start(ds[0:3])] + [step(ds[i:i + 3])
                                           for i in range(3, depth, 3)]

            def alloc_psc(h):
                # ctx accumulator: 4 p-tiles x (64 ctx cols + denom col).
                # All 4 groups share one PSUM bank, and a matmul with
                # start=True zeroes the whole bank -- so zero the tile once
                # and accumulate with start=False in every matmul.
                psc = psC.tile([128, NPT * (DH + 1)], f32, name=f"ps_c{h}",
                               tag="psC")
                nc.vector.memset(psc, 0.0)
                psc_of[h] = psc
                return psc

            def emit_scores(h, q, jt, rs):
                pss = psS.tile([128, 1024], f32, name=f"ps_s{h}_{q}",
                               tag="psS")
                for i in range(2):
                    st = 2 * q + i
                    nc.tensor.matmul(
                        pss[:, i * 512:(i + 1) * 512],
                        lhsT=kt_sb[jt][rs:rs + DH, st * 128:(st + 1) * 128],
                        rhs=qt_sb[jt][rs:rs + DH, :],
                        start=True, stop=True)
                et = expool.tile([128, 1024], bf16, name=f"et{h}_{q}",
                                 tag="et")
                nc.scalar.activation(et, pss, Exp, bias=0.0, scale=0.125)
                return et

            def emit_ctx(h, q, et, psc):
                for i in range(2):
                    st = 2 * q + i
                    for pt in range(NPT):
                        nc.tensor.matmul(
                            psc[:, pt * (DH + 1):(pt + 1) * (DH + 1)],
                            lhsT=et[:, i * 512 + pt * 128:
                                    i * 512 + (pt + 1) * 128],
                            rhs=vp_sb[st][:, h * (DH + 1):
                                          (h + 1) * (DH + 1)],
                            start=False, stop=(st == NST - 1))

            # ======================= heads =======================
            for h in range(H):
                jt = h // 2
                rs = (h % 2) * DH
                if h >= 1:
                    emit_normalize(h - 1)
                    if h % 2 == 0:
                        emit_transposes(jt - 1)
                psc = alloc_psc(h)
                # mid-head PE filler microtasks, spread evenly across the
                # quads to absorb the per-quad exp-latency bubble
                fillers = []
                if h in KT_CHUNKS:
                    jt_c, scs = KT_CHUNKS[h]
                    fillers += [(lambda j, s: lambda: emit_kt_sc(j, s))
                                (jt_c, sc) for sc in scs]
                for pt_o, jc_o, depth_o in OP1.get(h, ()):
                    fillers += oproj_p1_tasks(pt_o, jc_o, depth_o)
                for q in range(NG):
                    et = emit_scores(h, q, jt, rs)
                    k = -(-len(fillers) // (NG - q))
                    for _ in range(k):
                        fillers.pop(0)()
                    ensure_v(2 * q + 1)
                    emit_ctx(h, q, et, psc)
                while fillers:
                    fillers.pop(0)()

            # ======== tail: last-head normalize + jt7 transpose (on PE,
            # short latency) + o-proj pass 2 (d-tiles 6,7 + partial) ====
            Copy = mybir.ActivationFunctionType.Copy
            psc15 = psc_of[H - 1]
            # normalize-mul and transpose-copy chains split across DVE
            # (pt 0,1) and ACT (pt 2,3) so neither serializes the tail;
            # all psc15 reads complete before pst tiles recycle psC slots
            for pt in range(NPT):
                c0 = pt * (DH + 1)
                rc = rpool.tile([128, 1], f32, name=f"rc15_{pt}", tag="rc")
                nc.vector.reciprocal(rc, psc15[:, c0 + DH:c0 + DH + 1])
                if pt < 2:
                    nc.vector.tensor_scalar_mul(
                        ctx_pd[pt][:, (H - 1) * DH:H * DH],
                        psc15[:, c0:c0 + DH], rc)
                else:
                    nc.scalar.activation(
                        ctx_pd[pt][:, (H - 1) * DH:H * DH],
                        psc15[:, c0:c0 + DH], Copy, scale=rc)
            for pt in range(NPT):
                pst = psC.tile([128, 128], bf16, name=f"pst{pt}", tag="psC")
                nc.tensor.transpose(
                    pst, ctx_pd[pt][:, (NDT - 1) * 128:NDT * 128], idm_sb)
                if pt < 2:
                    nc.vector.tensor_copy(
                        cx_sb[NDT - 1][:, pt * 128:(pt + 1) * 128], pst)
                else:
                    nc.scalar.activation(
                        cx_sb[NDT - 1][:, pt * 128:(pt + 1) * 128], pst, Copy)

            for pt in range(NPT):
                ot = opool.tile([128, 1024], bf16, name=f"ot{pt}", tag="ot")
                for jc in range(2):
                    ps = psV.tile([128, 512], f32, name=f"ps_p2_{pt}_{jc}",
                                  tag="psV")
                    d0 = OP2[(pt, jc)]
                    for d in range(d0, NDT):
                        nc.tensor.matmul(
                            ps, lhsT=cx_sb[d][:, pt * 128:(pt + 1) * 128],
                            rhs=ow_sb[d][:, jc * 512:(jc + 1) * 512],
                            start=(d == d0), stop=(d == NDT - 1))
                    if pt >= 2:
                        # balance the serial merge-add chain: ACT converts
                        # the psum to bf16 (idle engine), making the DVE
                        # add eligible for the 2x 16-bit mode; split the
                        # writeback per half so the final DMA is not gated
                        # on both merge adds
                        psb = expool.tile([128, 1024], bf16,
                                          name=f"psb{pt}_{jc}", tag="et")
                        nc.scalar.activation(psb[:, 0:512], ps, Copy)
                        nc.vector.tensor_add(ot[:, jc * 512:(jc + 1) * 512],
                                             psb[:, 0:512], otp[(pt, jc)])
                        nc.sync.dma_start(
                            out=out.ap()[pt * 128:(pt + 1) * 128,
                                         jc * 512:(jc + 1) * 512],
                            in_=ot[:, jc * 512:(jc + 1) * 512])
                    else:
                        nc.vector.tensor_add(ot[:, jc * 512:(jc + 1) * 512],
                                             ps, otp[(pt, jc)])
                if pt < 2:
                    nc.sync.dma_start(
                        out=out.ap()[pt * 128:(pt + 1) * 128, :], in_=ot)


def get_nc():
    global _cached_nc
    if _cached_nc is None:
        _cached_nc = _build_nc()
    return _cached_nc


def make_in_maps(ts_features, llm_features, q_w, q_b, k_w, k_b, v_w, v_b,
                 o_w, o_b):
    ts = np.asarray(ts_features, np.float32)
    llm = np.asarray(llm_features, np.float32)
    shared = {
        "qwT": np.ascontiguousarray(np.asarray(q_w, np.float32).T).astype(_BF16),
        "kwT": np.ascontiguousarray(np.asarray(k_w, np.float32).T).astype(_BF16),
        "vwT": np.ascontiguousarray(np.asarray(v_w, np.float32).T).astype(_BF16),
        "owT": np.ascontiguousarray(np.asarray(o_w, np.float32).T).astype(_BF16),
        "qkb": np.ascontiguousarray(np.concatenate(
            [np.asarray(q_b, np.float32).reshape(NDT, 128).T,
             np.asarray(k_b, np.float32).reshape(NDT, 128).T], axis=1)),
        "idm": np.eye(128, dtype=np.float32).astype(_BF16),
        "vbb": np.ascontiguousarray(
            np.broadcast_to(np.asarray(v_b, np.float32), (128, D))).astype(_BF16),
        "obb": np.ascontiguousarray(
            np.broadcast_to(np.asarray(o_b, np.float32), (128, D))),
    }
    in_maps = []
    for b in range(NCORES):
        m = dict(shared)
        # ts2[r, d*512 + p] = ts[b].T[d*128 + r, p]
        m["ts2"] = np.ascontiguousarray(
            ts[b].T.reshape(NDT, 128, P).transpose(1, 0, 2)
            .reshape(128, NDT * P)).astype(_BF16)
        m["llmT"] = np.ascontiguousarray(llm[b].T).astype(_BF16)
        in_maps.append(m)
    return in_maps


def kernel(**inputs):
    from concourse.bass_utils import run_bass_kernel_spmd

    nc = get_nc()
    in_maps = make_in_maps(**inputs)
    res = run_bass_kernel_spmd(nc, in_maps, list(range(NCORES)))
    return np.stack([res.results[i]["out"] for i in range(NCORES)],
                    axis=0).astype(np.float32)
